# revision 1
# baseline (speedup 1.0000x reference)
"""Trainium2 Bass kernel for the MCRM block (4 local patches + global branch).

Sharding: 8 cores = 4 patches x 2 token-halves. Each core runs the full
attention+FFN pipeline for 4608 tokens of one patch in feature-major layout
(channels on partitions). The small global-branch work (saliency map, pooled
KV, glb output slice) is computed per-core on its slice.

Each core emits ONE merged DRAM output OALL[D, L+1152] (src tokens in columns
[0:L], the glb slice in [L:L+1152]). On the remote (axon-tunneled) execution
path every distinct output array costs a full client round-trip (~70 ms) in
addition to the per-call round-trip, independent of byte count, so merging
the two logical outputs halves the end-to-end executed-call latency.
"""
import sys
sys.path.insert(0, '/opt/trn_rl_repo')
import numpy as np

import concourse.bacc as bacc
import concourse.mybir as mybir
import concourse.tile as tile
from concourse.bass_utils import run_bass_kernel_spmd

F32 = mybir.dt.float32
F32R = mybir.dt.float32r
BF16 = mybir.dt.bfloat16
AF = mybir.ActivationFunctionType
OP = mybir.AluOpType

D = 384          # d_model
NH = 8           # heads
DH = 48          # head dim
DP = 64          # padded head dim
HW = 96
L = 4608         # tokens per core (half patch)
CH = 512         # token chunk
NCH = L // CH    # 9
S = 756          # kv tokens (576+144+36)
NST = 6          # s-tiles of 126
ST = 126
SCALE = 1.0 / np.sqrt(48.0)
EPS = 1e-5

_cache = {}


def _sel9():
    s = np.zeros((16, NCH * 128), np.float32)
    for ch in range(NCH):
        s[ch, 128 * ch:128 * (ch + 1)] = 1.0
    return s


def _build(use_g1, use_g2):
    nc = bacc.Bacc(target_bir_lowering=False, debug=False)

    def dparam(name, shape, dt=F32R):
        return nc.declare_dram_parameter(name, list(shape), dt, isOutput=False)

    X = dparam("X", (D, L))
    PG = dparam("PG", (D, 2304))
    GS = dparam("GS", (D, 1152))
    WQT = dparam("WQT", (D, 512))
    WKT = dparam("WKT", (D, 512))
    WVA = dparam("WVA", (D + 1, 512))
    WOT = dparam("WOT", (512, D), BF16)
    W3T = dparam("W3T", (D, 768))
    W4T = dparam("W4T", (768, D))
    SW = dparam("SW", (D, 1))
    ONESR = dparam("ONESR", (1, 756))
    ONESD = dparam("ONESD", (128, 1))          # value 1/384
    SEL9 = dparam("SEL9", (16, NCH * 128))     # one-hot row selectors
    BQ = dparam("BQ", (512, 1), F32)
    BK = dparam("BK", (512, 1), F32)
    BO = dparam("BO", (D, 1), F32)
    B3 = dparam("B3", (768, 1), F32)
    B4 = dparam("B4", (D, 1), F32)
    G1 = dparam("G1", (D, 1), F32)
    BE1 = dparam("BE1", (D, 1), F32)
    G2 = dparam("G2", (D, 1), F32)
    BE2 = dparam("BE2", (D, 1), F32)
    SALB = dparam("SALB", (1, 1), F32)
    # single merged output: columns [0:L] = src slice, [L:L+1152] = glb slice
    # (one output array per call: each extra output costs a full client
    # round-trip on the remote execution path)
    OALL = nc.declare_dram_parameter("OALL", [D, L + 1152], F32, isOutput=True)

    NP = 16  # partition count for stats tiles (>= NCH)

    with tile.TileContext(nc) as tc:
      with tc.tile_pool(name="const", bufs=1) as cp:
        onesd = cp.tile([128, 1], F32R)
        nc.sync.dma_start(onesd[:], ONESD[:])
        sel9 = cp.tile([NP, NCH * 128], F32R)
        nc.sync.dma_start(sel9[:], SEL9[:])
        salb = cp.tile([1, 1], F32)
        nc.sync.dma_start(salb[:], SALB[:])
        eps_t = cp.tile([NP, 1], F32)
        nc.vector.memset(eps_t[:], float(EPS))

        def col(par, n, tg):
            t = cp.tile([128, n // 128, 1], F32, tag=tg)
            nc.sync.dma_start(t[:], par.rearrange("(t p) o -> p t o", p=128))
            return t
        bq_c = col(BQ, 512, "bqc")
        bo_c = col(BO, D, "boc")
        b3_c = col(B3, 768, "b3c")
        b4_c = col(B4, D, "b4c")
        g1_c = col(G1, D, "g1c")
        be1_c = col(BE1, D, "be1c")
        g2_c = col(G2, D, "g2c")
        be2_c = col(BE2, D, "be2c")

        # per-chunk LN stats, one row per chunk (partition = chunk index)
        m19 = cp.tile([NP, CH], F32R)
        rs19 = cp.tile([NP, CH], F32R)
        m29 = cp.tile([NP, CH], F32R)
        rs29 = cp.tile([NP, CH], F32R)
        tamup = cp.tile([1, L], F32)
        for _s in (m19, rs19, m29, rs29):
            nc.vector.memset(_s[:].bitcast(F32), 0.0)

        with tc.tile_pool(name="bigA", bufs=1) as bpa:
          src = bpa.tile([128, 3, L], F32R, tag="bigA")
          with tc.tile_pool(name="apool", bufs=1) as apl:
            wq = apl.tile([128, 3, 512], F32R)
            nc.sync.dma_start(wq[:], WQT.rearrange("(t p) m -> p t m", p=128))
            wo = apl.tile([64, 8, D], BF16)
            nc.sync.dma_start(wo[:], WOT.rearrange("(h p) m -> p h m", p=64))
            k_sb = apl.tile([128, 4, S], F32R)
            v_sb = apl.tile([126, 6, 512], BF16)

            # ------------- setup: tam, pooling, K/V proj -------------
            with tc.tile_pool(name="setup", bufs=1) as spl, \
                 tc.tile_pool(name="setps", bufs=2, space="PSUM") as sps:
                wk = spl.tile([128, 3, 512], F32R)
                nc.sync.dma_start(wk[:], WKT.rearrange("(t p) m -> p t m", p=128))
                wv = spl.tile([128, 3, 512], F32R)
                nc.sync.dma_start(wv[:], WVA[0:D, :].rearrange("(t p) m -> p t m", p=128))
                wv1 = spl.tile([1, 512], F32R)
                nc.sync.dma_start(wv1[:], WVA[D:D + 1, :])
                sw = spl.tile([128, 3, 1], F32R)
                nc.sync.dma_start(sw[:], SW.rearrange("(t p) o -> p t o", p=128))
                onesr = spl.tile([1, 756], F32R)
                nc.sync.dma_start(onesr[:], ONESR[:])
                bk_c = spl.tile([128, 4, 1], F32)
                nc.sync.dma_start(bk_c[:], BK.rearrange("(t p) o -> p t o", p=128))
                gs = spl.tile([128, 3, 1152], F32R)
                nc.sync.dma_start(gs[:], GS.rearrange("(t p) m -> p t m", p=128))
                pg = spl.tile([128, 3, 2304], F32R)
                nc.sync.dma_start(pg[:], PG.rearrange("(t p) m -> p t m", p=128))

                tam = spl.tile([1, 1152], F32)
                for nchk in range(3):
                    tp = sps.tile([1, 384], F32, tag="tamps")
                    for t in range(3):
                        nc.tensor.matmul(tp[:], sw[:, t, :],
                                         gs[:, t, 384 * nchk:384 * (nchk + 1)],
                                         start=(t == 0), stop=(t == 2))
                    nc.scalar.activation(tam[:, 384 * nchk:384 * (nchk + 1)], tp[:],
                                         AF.Sigmoid, bias=salb[:, 0:1], scale=1.0)
                tview = tam[:].rearrange("o (r c) -> o r c", r=24)
                uview = tamup[:].rearrange("o (r a c b) -> o r a c b", r=24, a=2, b=2)
                for a in range(2):
                    for b in range(2):
                        nc.vector.tensor_copy(uview[:, :, a, :, b], tview[:])

                kv = spl.tile([128, 3, S], F32R)
                for t in range(3):
                    pgv = pg[:, t, :].rearrange("p (r a c) -> p r a c", r=24, a=2)
                    h1 = spl.tile([128, 24, 48], F32, tag="poolh")
                    nc.vector.tensor_tensor(h1[:], pgv[:, :, 0, :], pgv[:, :, 1, :],
                                            op=OP.add)
                    h1v = h1[:].rearrange("p r (c d) -> p r c d", d=2)
                    p4 = spl.tile([128, 24, 24], F32, tag="poolp4")
                    nc.vector.tensor_tensor(p4[:], h1v[:, :, :, 0], h1v[:, :, :, 1],
                                            op=OP.add)
                    nc.vector.tensor_scalar(kv[:, t, 0:576].rearrange(
                        "p (r c) -> p r c", r=24), p4[:], 0.25, None, op0=OP.mult)
                    k4 = kv[:, t, 0:576].rearrange("p (r a c) -> p r a c", r=12, a=2)
                    h2 = spl.tile([128, 12, 24], F32, tag="poolh2")
                    nc.vector.tensor_tensor(h2[:], k4[:, :, 0, :], k4[:, :, 1, :],
                                            op=OP.add)
                    h2v = h2[:].rearrange("p r (c d) -> p r c d", d=2)
                    p8 = spl.tile([128, 12, 12], F32, tag="poolp8")
                    nc.vector.tensor_tensor(p8[:], h2v[:, :, :, 0], h2v[:, :, :, 1],
                                            op=OP.add)
                    nc.vector.tensor_scalar(kv[:, t, 576:720].rearrange(
                        "p (r c) -> p r c", r=12), p8[:], 0.25, None, op0=OP.mult)
                    k8 = kv[:, t, 576:720].rearrange("p (r a c) -> p r a c", r=6, a=2)
                    h3 = spl.tile([128, 6, 12], F32, tag="poolh3")
                    nc.vector.tensor_tensor(h3[:], k8[:, :, 0, :], k8[:, :, 1, :],
                                            op=OP.add)
                    h3v = h3[:].rearrange("p r (c d) -> p r c d", d=2)
                    p16 = spl.tile([128, 6, 6], F32, tag="poolp16")
                    nc.vector.tensor_tensor(p16[:], h3v[:, :, :, 0], h3v[:, :, :, 1],
                                            op=OP.add)
                    nc.vector.tensor_scalar(kv[:, t, 720:756].rearrange(
                        "p (r c) -> p r c", r=6), p16[:], 0.25, None, op0=OP.mult)

                for mt in range(4):
                    for nchk in range(2):
                        nsl = slice(378 * nchk, 378 * (nchk + 1))
                        kp = sps.tile([128, 378], F32, tag="kvps")
                        for t in range(3):
                            nc.tensor.matmul(kp[:], wk[:, t, 128 * mt:128 * (mt + 1)],
                                             kv[:, t, nsl],
                                             start=(t == 0), stop=(t == 2))
                        nc.vector.tensor_scalar(k_sb[:, mt, nsl], kp[:],
                                                bk_c[:, mt, :], None, op0=OP.add)
                for st in range(NST):
                    ssl = slice(ST * st, ST * (st + 1))
                    vp = sps.tile([126, 512], F32, tag="kvps")
                    for t in range(3):
                        nc.tensor.matmul(vp[:], kv[:, t, ssl], wv[:, t, :],
                                         start=(t == 0), stop=False)
                    nc.tensor.matmul(vp[:], onesr[:, ssl], wv1[:],
                                     start=False, stop=True)
                    nc.vector.tensor_copy(v_sb[:, st, :], vp[:])

            # ------------- phase A: attention per chunk -------------
            with tc.tile_pool(name="wka", bufs=3) as wa, \
                 tc.tile_pool(name="qpool", bufs=5) as qpl, \
                 tc.tile_pool(name="epool", bufs=7) as ep, \
                 tc.tile_pool(name="aow", bufs=2) as aw, \
                 tc.tile_pool(name="aonp", bufs=9) as awn, \
                 tc.tile_pool(name="qps", bufs=2, space="PSUM") as qps, \
                 tc.tile_pool(name="scps", bufs=2, space="PSUM") as scps, \
                 tc.tile_pool(name="aops", bufs=2, space="PSUM") as aops:
                for ch in range(NCH):
                    csl = slice(CH * ch, CH * (ch + 1))
                    tb = wa.tile([128, CH], F32, tag="tb")
                    nc.gpsimd.partition_broadcast(tb[:], tamup[:, csl])
                    xm = wa.tile([128, 3, CH], F32R, tag="xc")
                    nc.sync.dma_start(
                        xm[:], X.rearrange("(t p) l -> p t l", p=128)[:, :, csl])
                    for t in range(3):
                        nc.vector.tensor_tensor(xm[:, t, :], xm[:, t, :], tb[:],
                                                op=OP.mult)
                    q = []
                    for mt in range(4):
                        qp = qps.tile([128, CH], F32, tag="qp")
                        for t in range(3):
                            nc.tensor.matmul(qp[:], wq[:, t, 128 * mt:128 * (mt + 1)],
                                             xm[:, t, :],
                                             start=(t == 0), stop=(t == 2))
                        qt = qpl.tile([128, CH], F32R, tag="q")
                        nc.vector.tensor_scalar(qt[:], qp[:],
                                                bq_c[:, mt, :], None, op0=OP.add)
                        q.append(qt)
                    for half in range(2):
                        e_t = []
                        for st in range(NST):
                            ssl = slice(ST * st, ST * (st + 1))
                            # scores psum split into 2-bank halves (bufs=2):
                            # double-buffers PE scores against ACT exp so the
                            # exp latency leaves the PE critical path
                            et = ep.tile([126, 4 * CH], BF16, tag="e")
                            for pp in range(2):
                                mt = 2 * half + pp
                                sp = scps.tile([126, 2 * CH], F32, tag="sc")
                                nc.tensor.matmul(
                                    sp[:, 0:CH],
                                    k_sb[0:64, mt, ssl], q[mt][0:64, :],
                                    start=True, stop=True, tile_position=(0, 0))
                                nc.tensor.matmul(
                                    sp[:, CH:2 * CH],
                                    k_sb[64:128, mt, ssl], q[mt][64:128, :],
                                    start=True, stop=True, tile_position=(64, 0))
                                nc.scalar.activation(
                                    et[:, CH * (2 * pp):CH * (2 * pp + 2)], sp[:],
                                    AF.Exp, scale=float(SCALE))
                            e_t.append(et)
                        for hh in range(4):
                            h = 4 * half + hh
                            ap_ = aops.tile([64, CH], F32, tag="ao")
                            for st in range(NST):
                                nc.tensor.matmul(
                                    ap_[:], v_sb[:, st, DP * h:DP * (h + 1)],
                                    e_t[st][:, CH * hh:CH * (hh + 1)],
                                    start=(st == 0), stop=(st == NST - 1))
                            # normalize straight out of PSUM (DVE reads PSUM)
                            # instead of staging through an SBUF copy
                            rc = aw.tile([1, CH], F32, tag="rowtmp")
                            nc.vector.reciprocal(rc[:], ap_[0:1, :])
                            bc = aw.tile([64, CH], F32, tag="bc")
                            nc.gpsimd.partition_broadcast(bc[:], rc[:])
                            an = awn.tile([64, CH], BF16, tag="aon")
                            nc.vector.tensor_tensor(an[:], ap_[:], bc[:], op=OP.mult)
                            if h == 0:
                                aon = []
                            aon.append(an)
                    for t in range(3):
                        op_ = qps.tile([128, CH], F32, tag="qp")
                        for h in range(NH):
                            nc.tensor.matmul(op_[:], wo[:, h, 128 * t:128 * (t + 1)],
                                             aon[h][:],
                                             start=(h == 0), stop=(h == NH - 1))
                        nc.vector.scalar_tensor_tensor(
                            src[:, t, csl], op_[:], bo_c[:, t, :], xm[:, t, :],
                            op0=OP.add, op1=OP.add)
                    # LN1 stats
                    sq = wa.tile([128, 3, CH], F32R, tag="xc")
                    for t in range(3):
                        nc.vector.tensor_tensor(sq[:, t, :], src[:, t, csl],
                                                src[:, t, csl], op=OP.mult)
                    stm = qps.tile([1, CH], F32, tag="qp")
                    for t in range(3):
                        nc.tensor.matmul(stm[:], onesd[:], src[:, t, csl],
                                         start=(t == 0), stop=(t == 2))
                    sts = qps.tile([1, CH], F32, tag="qp")
                    for t in range(3):
                        nc.tensor.matmul(sts[:], onesd[:], sq[:, t, :],
                                         start=(t == 0), stop=(t == 2))
                    mtr = aw.tile([1, CH], F32R, tag="rowtmp")
                    nc.vector.tensor_copy(mtr[:], stm[:])
                    nc.sync.dma_start(m19[ch:ch + 1, :], mtr[:])
                    msq = aw.tile([1, CH], F32, tag="rowtmp")
                    nc.vector.tensor_tensor(msq[:], mtr[:], mtr[:], op=OP.mult)
                    vtr = aw.tile([1, CH], F32, tag="rowtmp")
                    nc.vector.tensor_tensor(vtr[:], sts[:], msq[:], op=OP.subtract)
                    # store raw variance; rsqrt is batched over all chunks
                    # below so ACT keeps the Exp table loaded all of phase A
                    nc.gpsimd.dma_start(rs19[ch:ch + 1, :], vtr[:])
                # batched 1/sqrt(var+eps) for all 9 chunks in one 16-wide pass
                sva = aw.tile([NP, CH], F32, tag="svall")
                nc.scalar.activation(sva[:], rs19[:], AF.Sqrt,
                                     bias=eps_t[:, 0:1], scale=1.0)
                with nc.allow_low_precision(reason="f32r out is f32-width"):
                    nc.vector.reciprocal(rs19[:], sva[:])

          # ------------- phase C: LN1 apply + FFN -------------
          with tc.tile_pool(name="bigB", bufs=1) as bpb:
            src2 = bpb.tile([128, 3, L], F32R, tag="bigB")
            with tc.tile_pool(name="cpool", bufs=1) as cpl, \
                 tc.tile_pool(name="wkc", bufs=2) as wc, \
                 tc.tile_pool(name="cps", bufs=6, space="PSUM") as cps, \
                 tc.tile_pool(name="cbc", bufs=2, space="PSUM") as cbc:
                w3 = cpl.tile([128, 3, 768], F32R)
                nc.sync.dma_start(w3[:], W3T.rearrange("(t p) m -> p t m", p=128))
                w4 = cpl.tile([128, 6, D], F32R)
                nc.sync.dma_start(w4[:], W4T.rearrange("(t p) m -> p t m", p=128))
                for ch in range(NCH):
                    csl = slice(CH * ch, CH * (ch + 1))
                    ssel = sel9[:, 128 * ch:128 * (ch + 1)]
                    mb = cbc.tile([128, CH], F32, tag="cbc")
                    nc.tensor.matmul(mb[:], ssel, m19[:], start=True, stop=True)
                    rb = cbc.tile([128, CH], F32, tag="cbc")
                    nc.tensor.matmul(rb[:], ssel, rs19[:], start=True, stop=True)
                    sln = wc.tile([128, 3, CH], F32R, tag="sln")
                    for t in range(3):
                        ctr = wc.tile([128, CH], F32, tag="ctr")
                        nc.vector.tensor_tensor(ctr[:], src[:, t, csl], mb[:],
                                                op=OP.subtract)
                        nc.vector.tensor_tensor(sln[:, t, :], ctr[:], rb[:],
                                                op=OP.mult)
                        if use_g1:
                            nc.vector.tensor_scalar(sln[:, t, :], sln[:, t, :],
                                                    g1_c[:, t, :], be1_c[:, t, :],
                                                    op0=OP.mult, op1=OP.add)
                    hid = wc.tile([128, 6, CH], F32R, tag="hid")
                    for mt in range(6):
                        hp = cps.tile([128, CH], F32, tag="cp")
                        for t in range(3):
                            nc.tensor.matmul(hp[:], w3[:, t, 128 * mt:128 * (mt + 1)],
                                             sln[:, t, :],
                                             start=(t == 0), stop=(t == 2))
                        nc.scalar.activation(hid[:, mt, :], hp[:], AF.Gelu,
                                             bias=b3_c[:, mt, :], scale=1.0)
                    for t in range(3):
                        fp = cps.tile([128, CH], F32, tag="cp")
                        for kt in range(6):
                            nc.tensor.matmul(fp[:], w4[:, kt, 128 * t:128 * (t + 1)],
                                             hid[:, kt, :],
                                             start=(kt == 0), stop=(kt == 5))
                        nc.vector.scalar_tensor_tensor(
                            src2[:, t, csl], fp[:], b4_c[:, t, :], sln[:, t, :],
                            op0=OP.add, op1=OP.add)
                    sq2 = wc.tile([128, 3, CH], F32R, tag="sln")
                    for t in range(3):
                        nc.vector.tensor_tensor(sq2[:, t, :], src2[:, t, csl],
                                                src2[:, t, csl], op=OP.mult)
                    stm2 = cps.tile([1, CH], F32, tag="cp")
                    for t in range(3):
                        nc.tensor.matmul(stm2[:], onesd[:], src2[:, t, csl],
                                         start=(t == 0), stop=(t == 2))
                    sts2 = cps.tile([1, CH], F32, tag="cp")
                    for t in range(3):
                        nc.tensor.matmul(sts2[:], onesd[:], sq2[:, t, :],
                                         start=(t == 0), stop=(t == 2))
                    mtr2 = wc.tile([1, CH], F32R, tag="rowtmp2")
                    nc.vector.tensor_copy(mtr2[:], stm2[:])
                    nc.sync.dma_start(m29[ch:ch + 1, :], mtr2[:])
                    msq2 = wc.tile([1, CH], F32, tag="rowtmp2")
                    nc.vector.tensor_tensor(msq2[:], mtr2[:], mtr2[:], op=OP.mult)
                    vtr2 = wc.tile([1, CH], F32, tag="rowtmp2")
                    nc.vector.tensor_tensor(vtr2[:], sts2[:], msq2[:],
                                            op=OP.subtract)
                    # store raw variance; batched rsqrt below keeps the Gelu
                    # table loaded through phase C
                    nc.gpsimd.dma_start(rs29[ch:ch + 1, :], vtr2[:])
                sva2 = wc.tile([NP, CH], F32, tag="svall2")
                nc.scalar.activation(sva2[:], rs29[:], AF.Sqrt,
                                     bias=eps_t[:, 0:1], scale=1.0)
                with nc.allow_low_precision(reason="f32r out is f32-width"):
                    nc.vector.reciprocal(rs29[:], sva2[:])

            # ------------- phase E: LN2 apply, outputs -------------
            out = bpa.tile([128, 3, L], F32, tag="bigA")
            with tc.tile_pool(name="wke", bufs=3) as we, \
                 tc.tile_pool(name="egp", bufs=1) as eg, \
                 tc.tile_pool(name="ebc", bufs=2, space="PSUM") as ebc:
                for ch in range(NCH):
                    csl = slice(CH * ch, CH * (ch + 1))
                    ssel = sel9[:, 128 * ch:128 * (ch + 1)]
                    mb2 = ebc.tile([128, CH], F32, tag="ebc")
                    nc.tensor.matmul(mb2[:], ssel, m29[:], start=True, stop=True)
                    rb2 = ebc.tile([128, CH], F32, tag="ebc")
                    nc.tensor.matmul(rb2[:], ssel, rs29[:], start=True, stop=True)
                    for t in range(3):
                        ctr2 = we.tile([128, CH], F32, tag="ctr2")
                        nc.vector.tensor_tensor(ctr2[:], src2[:, t, csl], mb2[:],
                                                op=OP.subtract)
                        nc.vector.tensor_tensor(out[:, t, csl], ctr2[:], rb2[:],
                                                op=OP.mult)
                        if use_g2:
                            nc.vector.tensor_scalar(out[:, t, csl], out[:, t, csl],
                                                    g2_c[:, t, :], be2_c[:, t, :],
                                                    op0=OP.mult, op1=OP.add)
                nc.sync.dma_start(
                    OALL.rearrange("(t p) l -> p t l", p=128)[:, :, 0:L], out[:])
                gs2 = eg.tile([128, 3, 1152], F32R)
                nc.sync.dma_start(gs2[:], GS.rearrange("(t p) m -> p t m", p=128))
                gout = eg.tile([128, 3, 1152], F32, tag="gout")
                for t in range(3):
                    ov = out[:, t, :].rearrange("p (r a c b) -> p r a c b",
                                                r=24, a=2, b=2)
                    nc.vector.tensor_tensor(gout[:, t, :].rearrange(
                        "p (r c) -> p r c", r=24), gs2[:, t, :].rearrange(
                        "p (r c) -> p r c", r=24), ov[:, :, 0, :, 0], op=OP.add)
                nc.sync.dma_start(
                    OALL.rearrange("(t p) l -> p t l", p=128)[:, :, L:L + 1152],
                    gout[:])

    nc.finalize()
    return nc


def _prep_core(x, sal_w, sal_b, attn_in_w, attn_in_b, attn_out_w, attn_out_b,
               w3, b3, w4, b4, core):
    n, hf = core // 2, core % 2
    hg, wg = n // 2, n % 2
    f = np.float32
    Xc = np.ascontiguousarray(
        x[n, :, 48 * hf:48 * hf + 48, :].reshape(D, L), dtype=f)
    PGc = np.ascontiguousarray(
        x[4, :, 48 * hg:48 * hg + 48, 48 * wg:48 * wg + 48].reshape(D, 2304),
        dtype=f)
    r0 = 48 * hg + 24 * hf
    GSc = np.ascontiguousarray(
        x[4, :, r0:r0 + 24, 48 * wg:48 * wg + 48].reshape(D, 1152), dtype=f)

    wq = attn_in_w[n, 0:D]
    wk = attn_in_w[n, D:2 * D]
    wv = attn_in_w[n, 2 * D:3 * D]
    bq = attn_in_b[n, 0:D]
    bk = attn_in_b[n, D:2 * D]
    bv = attn_in_b[n, 2 * D:3 * D]

    WQT = np.zeros((D, 512), f)
    WKT = np.zeros((D, 512), f)
    BQc = np.zeros((512, 1), f)
    BKc = np.zeros((512, 1), f)
    WVA = np.zeros((D + 1, 512), f)
    WOT = np.zeros((512, D), f)
    for h in range(NH):
        WQT[:, DP * h:DP * h + DH] = wq[DH * h:DH * (h + 1), :].T
        WKT[:, DP * h:DP * h + DH] = wk[DH * h:DH * (h + 1), :].T
        BQc[DP * h:DP * h + DH, 0] = bq[DH * h:DH * (h + 1)]
        BKc[DP * h:DP * h + DH, 0] = bk[DH * h:DH * (h + 1)]
        WVA[0:D, DP * h + 1:DP * h + 1 + DH] = wv[DH * h:DH * (h + 1), :].T
        WVA[D, DP * h + 1:DP * h + 1 + DH] = bv[DH * h:DH * (h + 1)]
        WVA[D, DP * h] = 1.0
        WOT[DP * h + 1:DP * h + 1 + DH, :] = attn_out_w[n, :, DH * h:DH * (h + 1)].T
    return {
        "X": Xc, "PG": PGc, "GS": GSc,
        "WQT": WQT, "WKT": WKT, "WVA": WVA,
        "WOT": np.ascontiguousarray(WOT).astype(mybir.dt.np(BF16)),
        "W3T": np.ascontiguousarray(w3.T, dtype=f),
        "W4T": np.ascontiguousarray(w4.T, dtype=f),
        "SW": np.ascontiguousarray(sal_w.reshape(1, D).T, dtype=f),
        "ONESR": np.ones((1, 756), f),
        "ONESD": np.full((128, 1), 1.0 / D, f),
        "SEL9": _sel9(),
        "BQ": BQc, "BK": BKc,
        "BO": np.asarray(attn_out_b[n], f).reshape(D, 1),
        "B3": np.asarray(b3, f).reshape(768, 1),
        "B4": np.asarray(b4, f).reshape(D, 1),
        "G1": None, "BE1": None, "G2": None, "BE2": None,
        "SALB": np.asarray(sal_b, f).reshape(1, 1),
    }


def kernel(x, sal_w, sal_b, attn_in_w, attn_in_b, attn_out_w, attn_out_b,
           w3, b3, w4, b4, g1, be1, g2, be2):
    f = np.float32
    x = np.asarray(x, f)
    args = [np.asarray(a, f) for a in
            (sal_w, sal_b, attn_in_w, attn_in_b, attn_out_w, attn_out_b,
             w3, b3, w4, b4)]
    g1, be1, g2, be2 = (np.asarray(a, f) for a in (g1, be1, g2, be2))
    use_g1 = not (np.all(g1 == 1.0) and np.all(be1 == 0.0))
    use_g2 = not (np.all(g2 == 1.0) and np.all(be2 == 0.0))

    key = (use_g1, use_g2)
    if key not in _cache:
        _cache[key] = _build(use_g1, use_g2)
    nc = _cache[key]

    in_maps = []
    for core in range(8):
        m = _prep_core(x, *args, core)
        m["G1"] = g1.reshape(D, 1)
        m["BE1"] = be1.reshape(D, 1)
        m["G2"] = g2.reshape(D, 1)
        m["BE2"] = be2.reshape(D, 1)
        in_maps.append(m)

    res = run_bass_kernel_spmd(nc, in_maps, list(range(8)))

    out = np.empty((5, D, HW, HW), f)
    for core in range(8):
        n, hf = core // 2, core % 2
        hg, wg = n // 2, n % 2
        oall = res.results[core]["OALL"]
        osrc = oall[:, 0:L].reshape(D, 48, 96)
        out[n, :, 48 * hf:48 * hf + 48, :] = osrc
        oglb = oall[:, L:L + 1152].reshape(D, 24, 48)
        r0 = 48 * hg + 24 * hf
        out[4, :, r0:r0 + 24, 48 * wg:48 * wg + 48] = oglb
    return out



# revision 36
# speedup vs baseline: 139.1037x; 139.1037x over previous
"""Trainium2 Bass kernel for the MCRM block (4 local patches + global branch).

Sharding: 8 cores = 4 patches x 2 token-halves. Each core runs the full
attention+FFN pipeline for 4608 tokens of one patch in feature-major layout
(channels on partitions). The small global-branch work (saliency map, pooled
KV, glb output slice) is computed per-core on its slice.

Each core emits ONE merged DRAM output OALL[D, L+1152] (src tokens in columns
[0:L], the glb slice in [L:L+1152]). On the remote (axon-tunneled) execution
path every distinct output array costs a full client round-trip (~70 ms) in
addition to the per-call round-trip, independent of byte count, so merging
the two logical outputs halves the end-to-end executed-call latency.
"""
import sys
sys.path.insert(0, '/opt/trn_rl_repo')
import numpy as np

import concourse.bacc as bacc
import concourse.mybir as mybir
import concourse.tile as tile
from concourse.bass_utils import run_bass_kernel_spmd

F32 = mybir.dt.float32
F32R = mybir.dt.float32r
BF16 = mybir.dt.bfloat16
AF = mybir.ActivationFunctionType
OP = mybir.AluOpType

D = 384          # d_model
NH = 8           # heads
DH = 48          # head dim
DP = 64          # padded head dim
HW = 96
L = 4608         # tokens per core (half patch)
CH = 512         # token chunk
NCH = L // CH    # 9
S = 756          # kv tokens (576+144+36)
NST = 6          # s-tiles of 126
ST = 126
SCALE = 1.0 / np.sqrt(48.0)
EPS = 1e-5

_cache = {}


def _sel9():
    s = np.zeros((16, NCH * 128), np.float32)
    for ch in range(NCH):
        s[ch, 128 * ch:128 * (ch + 1)] = 1.0
    return s


def _build(use_g1, use_g2):
    nc = bacc.Bacc(target_bir_lowering=False, debug=False)

    def dparam(name, shape, dt=F32R):
        return nc.declare_dram_parameter(name, list(shape), dt, isOutput=False)

    X = dparam("X", (D, L))
    PG = dparam("PG", (D, 2304), BF16)
    GS = dparam("GS", (D, 1152))
    WQT = dparam("WQT", (D, 512))
    WKT = dparam("WKT", (D, 512))
    WVA = dparam("WVA", (D + 1, 512))
    WOT = dparam("WOT", (512, D), BF16)
    W3T = dparam("W3T", (D, 768))
    W4T = dparam("W4T", (768, D), BF16)
    SW = dparam("SW", (D, 1))
    ONESR = dparam("ONESR", (1, 756))
    ONESD = dparam("ONESD", (128, 1))          # value 1/384
    ONESDB = dparam("ONESDB", (128, 1), BF16)  # value 1/384
    SEL9 = dparam("SEL9", (16, NCH * 128))     # one-hot row selectors
    BQ = dparam("BQ", (512, 1), F32)
    BK = dparam("BK", (512, 1), F32)
    BO = dparam("BO", (D, 1), F32)
    B3 = dparam("B3", (768, 1), F32)
    B4 = dparam("B4", (D, 1), F32)
    G1 = dparam("G1", (D, 1), F32)
    BE1 = dparam("BE1", (D, 1), F32)
    G2 = dparam("G2", (D, 1), F32)
    BE2 = dparam("BE2", (D, 1), F32)
    SALB = dparam("SALB", (1, 1), F32)
    # single merged output: columns [0:L] = src slice, [L:L+1152] = glb slice
    # (one output array per call: each extra output costs a full client
    # round-trip on the remote execution path)
    OALL = nc.declare_dram_parameter("OALL", [D, L + 1152], F32, isOutput=True)

    NP = 16  # partition count for stats tiles (>= NCH)

    with tile.TileContext(nc) as tc:
      with tc.tile_pool(name="const", bufs=1) as cp:
        onesd = cp.tile([128, 1], F32R)
        nc.sync.dma_start(onesd[:], ONESD[:])
        onesdb = cp.tile([128, 1], BF16)
        nc.sync.dma_start(onesdb[:], ONESDB[:])
        sel9 = cp.tile([NP, NCH * 128], F32R)
        nc.sync.dma_start(sel9[:], SEL9[:])
        salb = cp.tile([1, 1], F32)
        nc.sync.dma_start(salb[:], SALB[:])
        eps_t = cp.tile([NP, 1], F32)
        nc.vector.memset(eps_t[:], float(EPS))

        def col(par, n, tg):
            t = cp.tile([128, n // 128, 1], F32, tag=tg)
            nc.sync.dma_start(t[:], par.rearrange("(t p) o -> p t o", p=128))
            return t
        bq_c = col(BQ, 512, "bqc")
        bo_c = col(BO, D, "boc")
        b3_c = col(B3, 768, "b3c")
        b4_c = col(B4, D, "b4c")
        g1_c = col(G1, D, "g1c")
        be1_c = col(BE1, D, "be1c")
        g2_c = col(G2, D, "g2c")
        be2_c = col(BE2, D, "be2c")

        # per-chunk LN stats, one row per chunk (partition = chunk index)
        m19 = cp.tile([NP, CH], F32R)
        rs19 = cp.tile([NP, CH], F32R)
        m29 = cp.tile([NP, CH], F32R)
        rs29 = cp.tile([NP, CH], F32R)
        tamup = cp.tile([1, L], F32)
        for _s in (m19, rs19, m29, rs29):
            nc.vector.memset(_s[:].bitcast(F32), 0.0)

        with tc.tile_pool(name="bigA", bufs=1) as bpa:
          src = bpa.tile([128, 3, L], F32R, tag="bigA")
          with tc.tile_pool(name="apool", bufs=1) as apl:
            wq = apl.tile([128, 3, 512], F32R)
            nc.sync.dma_start(wq[:], WQT.rearrange("(t p) m -> p t m", p=128))
            wo = apl.tile([128, 4, D], BF16)
            nc.sync.dma_start(wo[:], WOT.rearrange("(h p) m -> p h m", p=128))
            k_sb = apl.tile([128, 4, S], F32R)
            v_sb = apl.tile([126, 6, 512], BF16)

            # ------------- setup: tam, pooling, K/V proj -------------
            with tc.tile_pool(name="setup", bufs=1) as spl, \
                 tc.tile_pool(name="setps", bufs=2, space="PSUM") as sps:
                wk = spl.tile([128, 3, 512], F32R)
                nc.sync.dma_start(wk[:], WKT.rearrange("(t p) m -> p t m", p=128))
                wv = spl.tile([128, 3, 512], F32R)
                nc.sync.dma_start(wv[:], WVA[0:D, :].rearrange("(t p) m -> p t m", p=128))
                wv1 = spl.tile([1, 512], F32R)
                nc.sync.dma_start(wv1[:], WVA[D:D + 1, :])
                sw = spl.tile([128, 3, 1], F32R)
                nc.sync.dma_start(sw[:], SW.rearrange("(t p) o -> p t o", p=128))
                onesr = spl.tile([1, 756], F32R)
                nc.sync.dma_start(onesr[:], ONESR[:])
                bk_c = spl.tile([128, 4, 1], F32)
                nc.sync.dma_start(bk_c[:], BK.rearrange("(t p) o -> p t o", p=128))
                gs = spl.tile([128, 3, 1152], F32R)
                nc.sync.dma_start(gs[:], GS.rearrange("(t p) m -> p t m", p=128))
                pg = spl.tile([128, 3, 2304], BF16)
                nc.sync.dma_start(pg[:], PG.rearrange("(t p) m -> p t m", p=128))

                uview = tamup[:].rearrange("o (r a c b) -> o r a c b", r=24, a=2, b=2)
                for nchk in range(3):
                    tp = sps.tile([1, 384], F32, tag="tamps")
                    for t in range(3):
                        nc.tensor.matmul(tp[:], sw[:, t, :],
                                         gs[:, t, 384 * nchk:384 * (nchk + 1)],
                                         start=(t == 0), stop=(t == 2))
                    tamc = spl.tile([1, 384], F32, tag="tamc", bufs=2,
                                    name=f"tamc_{nchk}")
                    nc.scalar.activation(tamc[:], tp[:],
                                         AF.Sigmoid, bias=salb[:, 0:1], scale=1.0)
                    tcv = tamc[:].rearrange("o (r c) -> o r c", r=8)
                    for a in range(2):
                        for b in range(2):
                            nc.vector.tensor_copy(
                                uview[:, 8 * nchk:8 * (nchk + 1), a, :, b], tcv[:])

                kv = spl.tile([128, 3, S], F32R)
                for t in range(3):
                    pgv = pg[:, t, :].rearrange("p (r a c) -> p r a c", r=24, a=2)
                    h1 = spl.tile([128, 24, 48], F32, tag="poolh")
                    nc.vector.tensor_tensor(h1[:], pgv[:, :, 0, :], pgv[:, :, 1, :],
                                            op=OP.add)
                    h1v = h1[:].rearrange("p r (c d) -> p r c d", d=2)
                    p4 = spl.tile([128, 24, 24], F32, tag="poolp4")
                    nc.vector.tensor_tensor(p4[:], h1v[:, :, :, 0], h1v[:, :, :, 1],
                                            op=OP.add)
                    nc.vector.tensor_scalar(kv[:, t, 0:576].rearrange(
                        "p (r c) -> p r c", r=24), p4[:], 0.25, None, op0=OP.mult)
                    k4 = kv[:, t, 0:576].rearrange("p (r a c) -> p r a c", r=12, a=2)
                    h2 = spl.tile([128, 12, 24], F32, tag="poolh2")
                    nc.vector.tensor_tensor(h2[:], k4[:, :, 0, :], k4[:, :, 1, :],
                                            op=OP.add)
                    h2v = h2[:].rearrange("p r (c d) -> p r c d", d=2)
                    p8 = spl.tile([128, 12, 12], F32, tag="poolp8")
                    nc.vector.tensor_tensor(p8[:], h2v[:, :, :, 0], h2v[:, :, :, 1],
                                            op=OP.add)
                    nc.vector.tensor_scalar(kv[:, t, 576:720].rearrange(
                        "p (r c) -> p r c", r=12), p8[:], 0.25, None, op0=OP.mult)
                    k8 = kv[:, t, 576:720].rearrange("p (r a c) -> p r a c", r=6, a=2)
                    h3 = spl.tile([128, 6, 12], F32, tag="poolh3")
                    nc.vector.tensor_tensor(h3[:], k8[:, :, 0, :], k8[:, :, 1, :],
                                            op=OP.add)
                    h3v = h3[:].rearrange("p r (c d) -> p r c d", d=2)
                    p16 = spl.tile([128, 6, 6], F32, tag="poolp16")
                    nc.vector.tensor_tensor(p16[:], h3v[:, :, :, 0], h3v[:, :, :, 1],
                                            op=OP.add)
                    nc.vector.tensor_scalar(kv[:, t, 720:756].rearrange(
                        "p (r c) -> p r c", r=6), p16[:], 0.25, None, op0=OP.mult)

                for mt in range(4):
                    for nchk in range(2):
                        nsl = slice(378 * nchk, 378 * (nchk + 1))
                        kp = sps.tile([128, 378], F32, tag="kvps")
                        for t in range(3):
                            nc.tensor.matmul(kp[:], wk[:, t, 128 * mt:128 * (mt + 1)],
                                             kv[:, t, nsl],
                                             start=(t == 0), stop=(t == 2))
                        nc.vector.tensor_scalar(k_sb[:, mt, nsl], kp[:],
                                                bk_c[:, mt, :], None, op0=OP.add)
                for st in range(NST):
                    ssl = slice(ST * st, ST * (st + 1))
                    vp = sps.tile([126, 512], F32, tag="kvps")
                    for t in range(3):
                        nc.tensor.matmul(vp[:], kv[:, t, ssl], wv[:, t, :],
                                         start=(t == 0), stop=False)
                    nc.tensor.matmul(vp[:], onesr[:, ssl], wv1[:],
                                     start=False, stop=True)
                    nc.vector.tensor_copy(v_sb[:, st, :], vp[:])

            # ------------- phase A: attention, software-pipelined -------------
            # Per iteration ch the engine queues get:
            #   PE : Q_ch | WO+stats(ch-2) | scores+AV_ch | sts(ch-2)
            #   DVE: mask_{ch+1} qbias_ch residual/sq_{ch-2} norm_ch stats_{ch-2}
            # so the softmax-normalize chain (recip+bcast+mult) of chunk ch
            # resolves during the next chunk's scores/AV and never stalls PE.
            with tc.tile_pool(name="wka", bufs=4) as wa, \
                 tc.tile_pool(name="tbp", bufs=2) as tbp, \
                 tc.tile_pool(name="qpool", bufs=4) as qpl, \
                 tc.tile_pool(name="epool", bufs=6) as ep, \
                 tc.tile_pool(name="aow", bufs=2) as aw, \
                 tc.tile_pool(name="aonp", bufs=9) as awn, \
                 tc.tile_pool(name="sqp", bufs=2) as sqp, \
                 tc.tile_pool(name="bigps", bufs=2, space="PSUM") as bps, \
                 tc.tile_pool(name="avps", bufs=3, space="PSUM") as avp:

                def emit_xdma(ch):
                    csl = slice(CH * ch, CH * (ch + 1))
                    xm = wa.tile([128, 3, CH], F32R, tag="xc")
                    nc.sync.dma_start(
                        xm[:], X.rearrange("(t p) l -> p t l", p=128)[:, :, csl])
                    return xm

                def emit_mask(ch, xm):
                    csl = slice(CH * ch, CH * (ch + 1))
                    tb = tbp.tile([128, CH], F32, tag="tb")
                    nc.gpsimd.partition_broadcast(tb[:], tamup[:, csl])
                    for t in range(3):
                        nc.vector.tensor_tensor(xm[:, t, :], xm[:, t, :], tb[:],
                                                op=OP.mult)

                def emit_q(ch, xm):
                    q = []
                    for mt in range(4):
                        qp = bps.tile([128, 2 * CH], F32, tag="ps")
                        for t in range(3):
                            nc.tensor.matmul(qp[:, 0:CH],
                                             wq[:, t, 128 * mt:128 * (mt + 1)],
                                             xm[:, t, :],
                                             start=(t == 0), stop=(t == 2))
                        qt = qpl.tile([128, CH], F32R, tag="q")
                        nc.vector.tensor_scalar(qt[:], qp[:, 0:CH],
                                                bq_c[:, mt, :], None, op0=OP.add)
                        q.append(qt)
                    return q

                def emit_attn(ch, q):
                    aon = []
                    for mt in range(4):          # head pair (heads 2mt, 2mt+1)
                        e_t = []
                        for st in range(NST):
                            ssl = slice(ST * st, ST * (st + 1))
                            sp = bps.tile([128, 2 * CH], F32, tag="ps")
                            nc.tensor.matmul(
                                sp[0:126, 0:CH],
                                k_sb[0:64, mt, ssl], q[mt][0:64, :],
                                start=True, stop=True, tile_position=(0, 0))
                            nc.tensor.matmul(
                                sp[0:126, CH:2 * CH],
                                k_sb[64:128, mt, ssl], q[mt][64:128, :],
                                start=True, stop=True, tile_position=(64, 0))
                            et = ep.tile([126, 2 * CH], BF16, tag="e")
                            nc.scalar.activation(et[:], sp[0:126, :],
                                                 AF.Exp, scale=float(SCALE))
                            e_t.append(et)
                        # both heads of the pair accumulate into one PSUM bank:
                        # rows 0:64 head 2mt, rows 64:128 head 2mt+1
                        ap_ = avp.tile([128, CH], F32, tag="av")
                        for st in range(NST):
                            nc.tensor.matmul(
                                ap_[0:64, :],
                                v_sb[:, st, 128 * mt:128 * mt + 64],
                                e_t[st][:, 0:CH],
                                start=(st == 0), stop=(st == NST - 1),
                                tile_position=(0, 0))
                            nc.tensor.matmul(
                                ap_[64:128, :],
                                v_sb[:, st, 128 * mt + 64:128 * (mt + 1)],
                                e_t[st][:, CH:2 * CH],
                                start=(st == 0), stop=(st == NST - 1),
                                tile_position=(0, 64))
                        rc0 = aw.tile([1, CH], F32, tag="rowtmp")
                        nc.vector.reciprocal_approx_fast(rc0[:], ap_[0:1, :])
                        rc1 = aw.tile([1, CH], F32, tag="rowtmp")
                        nc.vector.reciprocal_approx_fast(rc1[:], ap_[64:65, :])
                        # partition_broadcast only lands correctly on ranges
                        # based at partition 0 — broadcast each denominator to
                        # a full tile and multiply the matching lane halves
                        bca = aw.tile([128, CH], F32, tag="bc")
                        nc.gpsimd.partition_broadcast(bca[:], rc0[:])
                        bcb = aw.tile([128, CH], F32, tag="bc")
                        nc.gpsimd.partition_broadcast(bcb[:], rc1[:])
                        an = awn.tile([128, CH], BF16, tag="aon")
                        nc.vector.tensor_tensor(an[0:64, :], ap_[0:64, :],
                                                bca[0:64, :], op=OP.mult)
                        nc.vector.tensor_tensor(an[64:128, :], ap_[64:128, :],
                                                bcb[64:128, :], op=OP.mult)
                        aon.append(an)
                    return aon

                def emit_b1(ch, xm, aon):
                    # out-projection (contraction 128 = head pair) + residual
                    csl = slice(CH * ch, CH * (ch + 1))
                    for t in range(3):
                        op_ = bps.tile([128, 2 * CH], F32, tag="ps")
                        for mt in range(4):
                            nc.tensor.matmul(op_[:, 0:CH],
                                             wo[:, mt, 128 * t:128 * (t + 1)],
                                             aon[mt][:],
                                             start=(mt == 0), stop=(mt == 3))
                        nc.vector.scalar_tensor_tensor(
                            src[:, t, csl], op_[:, 0:CH], bo_c[:, t, :],
                            xm[:, t, :], op0=OP.add, op1=OP.add)
                    stm = bps.tile([128, 2 * CH], F32, tag="ps")
                    for t in range(3):
                        nc.tensor.matmul(stm[0:1, 0:CH], onesd[:], src[:, t, csl],
                                         start=(t == 0), stop=(t == 2))
                    mrow = aw.tile([1, CH], F32R, tag="mrow", bufs=2)
                    nc.vector.tensor_copy(mrow[:], stm[0:1, 0:CH])
                    nc.sync.dma_start(m19[ch:ch + 1, :], mrow[:])
                    sq = []
                    for t in range(3):
                        sqt = sqp.tile([128, CH], F32R, tag="sq", bufs=4)
                        nc.vector.tensor_tensor(sqt[:], src[:, t, csl],
                                                src[:, t, csl], op=OP.mult)
                        sq.append(sqt)
                    return sq

                def emit_b2(ch, sq):
                    sts = bps.tile([128, 2 * CH], F32, tag="ps")
                    for t in range(3):
                        nc.tensor.matmul(sts[0:1, 0:CH], onesd[:], sq[t][:],
                                         start=(t == 0), stop=(t == 2))
                    vrow = aw.tile([1, CH], F32R, tag="mrow", bufs=2)
                    nc.vector.tensor_copy(vrow[:], sts[0:1, 0:CH])
                    nc.gpsimd.dma_start(rs19[ch:ch + 1, :], vrow[:])

                st_xm = {0: emit_xdma(0), 1: emit_xdma(1)}
                emit_mask(0, st_xm[0])
                st_aon, st_b = {}, {}
                for ch in range(NCH):
                    if ch + 2 < NCH:
                        st_xm[ch + 2] = emit_xdma(ch + 2)
                    if ch + 1 < NCH:
                        emit_mask(ch + 1, st_xm[ch + 1])
                    q = emit_q(ch, st_xm[ch])
                    if ch >= 2:
                        st_b[ch - 2] = emit_b1(ch - 2, st_xm[ch - 2],
                                               st_aon.pop(ch - 2))
                        del st_xm[ch - 2]
                    st_aon[ch] = emit_attn(ch, q)
                    if ch >= 3:
                        emit_b2(ch - 3, st_b.pop(ch - 3))
                st_b[NCH - 2] = emit_b1(NCH - 2, st_xm.pop(NCH - 2),
                                        st_aon.pop(NCH - 2))
                emit_b2(NCH - 3, st_b.pop(NCH - 3))
                st_b[NCH - 1] = emit_b1(NCH - 1, st_xm.pop(NCH - 1),
                                        st_aon.pop(NCH - 1))
                emit_b2(NCH - 2, st_b.pop(NCH - 2))
                emit_b2(NCH - 1, st_b.pop(NCH - 1))

                # batched var = E[x^2] - mean^2 then 1/sqrt(var+eps),
                # all 9 chunks in one 16-wide pass
                msq = aw.tile([NP, CH], F32, tag="svall")
                nc.vector.tensor_tensor(msq[:], m19[:], m19[:], op=OP.mult)
                nc.vector.tensor_tensor(msq[:], rs19[:].bitcast(F32), msq[:],
                                        op=OP.subtract)
                sva = aw.tile([NP, CH], F32, tag="svall")
                nc.scalar.activation(sva[:], msq[:], AF.Sqrt,
                                     bias=eps_t[:, 0:1], scale=1.0)
                svb = aw.tile([NP, CH], F32, tag="svall")
                nc.vector.reciprocal_approx_fast(svb[:], sva[:])
                nc.vector.tensor_copy(rs19[:], svb[:])

          # ------- phase C+E fused: LN1+FFN+LN2+store, software-pipelined -------
          # Per iteration: PE gets mb/rb+W3(ch) | W4+stats2(ch-1); the LN2 of
          # chunk ch-2 (per-chunk sqrt/recip + gpsimd broadcasts + DVE apply)
          # resolves under the next chunks' matmuls. Output slices stream to
          # DRAM per chunk; the glb branch accumulates per-chunk into gout.
          with tc.tile_pool(name="cpool", bufs=1) as cpl, \
               tc.tile_pool(name="wkc", bufs=2) as wc, \
               tc.tile_pool(name="srmurow", bufs=3) as srp, \
               tc.tile_pool(name="cps", bufs=6, space="PSUM") as cps, \
               tc.tile_pool(name="cbc", bufs=2, space="PSUM") as cbc:
            w3 = cpl.tile([128, 3, 768], F32R)
            nc.sync.dma_start(w3[:], W3T.rearrange("(t p) m -> p t m", p=128))
            w4 = cpl.tile([128, 6, D], BF16)
            nc.sync.dma_start(w4[:], W4T.rearrange("(t p) m -> p t m", p=128))
            gout = cpl.tile([128, 3, 1152], F32R)
            nc.sync.dma_start(gout[:], GS.rearrange("(t p) m -> p t m", p=128))

            def emit_c1(ch):
                # LN1 apply + W3 + gelu
                csl = slice(CH * ch, CH * (ch + 1))
                ssel = sel9[:, 128 * ch:128 * (ch + 1)]
                mb = cbc.tile([128, CH], F32, tag="cbc")
                nc.tensor.matmul(mb[:], ssel, m19[:], start=True, stop=True)
                rb = cbc.tile([128, CH], F32, tag="cbc")
                nc.tensor.matmul(rb[:], ssel, rs19[:], start=True, stop=True)
                sln = wc.tile([128, 3, CH], F32R, tag="sln")
                for t in range(3):
                    ctr = wc.tile([128, CH], F32, tag="ctr")
                    nc.vector.tensor_tensor(ctr[:], src[:, t, csl], mb[:],
                                            op=OP.subtract)
                    nc.vector.tensor_tensor(sln[:, t, :], ctr[:], rb[:],
                                            op=OP.mult)
                    if use_g1:
                        nc.vector.tensor_scalar(sln[:, t, :], sln[:, t, :],
                                                g1_c[:, t, :], be1_c[:, t, :],
                                                op0=OP.mult, op1=OP.add)
                hid = wc.tile([128, 6, CH], BF16, tag="hid")
                for mt in range(6):
                    hp = cps.tile([128, CH], F32, tag="cp")
                    for t in range(3):
                        nc.tensor.matmul(hp[:], w3[:, t, 128 * mt:128 * (mt + 1)],
                                         sln[:, t, :],
                                         start=(t == 0), stop=(t == 2))
                    nc.scalar.activation(hid[:, mt, :], hp[:], AF.Gelu,
                                         bias=b3_c[:, mt, :], scale=1.0)
                return sln, hid

            def emit_c2a(ch, sln, hid):
                # W4 + residual + LN2 mean; squares go to gpsimd
                s2 = srp.tile([128, 3, CH], BF16, tag="s2", bufs=3)
                for t in range(3):
                    fp = cps.tile([128, CH], F32, tag="cp")
                    for kt in range(6):
                        nc.tensor.matmul(fp[:], w4[:, kt, 128 * t:128 * (t + 1)],
                                         hid[:, kt, :],
                                         start=(kt == 0), stop=(kt == 5))
                    nc.vector.scalar_tensor_tensor(
                        s2[:, t, :], fp[:], b4_c[:, t, :], sln[:, t, :],
                        op0=OP.add, op1=OP.add)
                stm2 = cps.tile([128, CH], F32, tag="cp")
                for t in range(3):
                    nc.tensor.matmul(stm2[0:1, :], onesdb[:], s2[:, t, :],
                                     start=(t == 0), stop=(t == 2))
                mrow = srp.tile([1, CH], F32, tag="mrow")
                nc.vector.tensor_copy(mrow[:], stm2[0:1, :])
                sq2l = []
                for t in range(3):
                    sq2 = wc.tile([128, CH], BF16, tag="sq2", bufs=6)
                    nc.vector.tensor_tensor(sq2[:], s2[:, t, :], s2[:, t, :],
                                            op=OP.mult)
                    sq2l.append(sq2)
                return s2, mrow, sq2l

            def emit_c2b(ch, s2, mrow, sq2l):
                sts2 = cps.tile([128, CH], F32, tag="cp")
                for t in range(3):
                    nc.tensor.matmul(sts2[0:1, :], onesdb[:], sq2l[t][:],
                                     start=(t == 0), stop=(t == 2))
                vrow = srp.tile([1, CH], F32, tag="vrow", bufs=2)
                nc.vector.tensor_tensor(vrow[:], mrow[:], mrow[:], op=OP.mult)
                nc.vector.tensor_tensor(vrow[:], sts2[0:1, :], vrow[:],
                                        op=OP.subtract)
                svr = srp.tile([1, CH], F32, tag="vrow", bufs=2)
                nc.scalar.activation(svr[:], vrow[:], AF.Sqrt,
                                     bias=eps_t[0:1, 0:1], scale=1.0)
                rrow = srp.tile([1, CH], F32, tag="rrow")
                nc.vector.reciprocal_approx_fast(rrow[:], svr[:])
                return s2, mrow, rrow

            ot_tiles = {}

            def emit_e(ch, s2, mrow, rrow):
                # LN2 apply via gpsimd broadcasts + store the chunk
                csl = slice(CH * ch, CH * (ch + 1))
                mb2 = srp.tile([128, CH], F32, tag="bc2")
                nc.gpsimd.partition_broadcast(mb2[:], mrow[:])
                rb2 = srp.tile([128, CH], F32, tag="bc2")
                nc.gpsimd.partition_broadcast(rb2[:], rrow[:])
                ot = srp.tile([128, 3, CH], F32, tag="ot")
                ot_tiles[ch] = ot
                for t in range(3):
                    ctr2 = wc.tile([128, CH], F32, tag="ctr2")
                    nc.vector.tensor_tensor(ctr2[:], s2[:, t, :], mb2[:],
                                            op=OP.subtract)
                    nc.vector.tensor_tensor(ot[:, t, :], ctr2[:], rb2[:],
                                            op=OP.mult)
                    if use_g2:
                        nc.vector.tensor_scalar(ot[:, t, :], ot[:, t, :],
                                                g2_c[:, t, :], be2_c[:, t, :],
                                                op0=OP.mult, op1=OP.add)
                nc.sync.dma_start(
                    OALL.rearrange("(t p) l -> p t l", p=128)[:, :, csl],
                    ot[:])
                # glb accumulation: even rows fully stored by now, in-place
                # add their even columns into gout (rows may straddle chunks)
                lim = CH * (ch + 1)
                r = emit_e.next_row
                while 96 * (r + 1) <= lim:
                    t0, t1 = 96 * r, 96 * (r + 1)
                    segs = []
                    cs = t0 // CH
                    if t1 - 1 >= CH * (cs + 1):
                        m = CH * (cs + 1)
                        segs = [(cs, t0, m), (cs + 1, m, t1)]
                    else:
                        segs = [(cs, t0, t1)]
                    for (c_, a, b) in segs:
                        k0, k1 = (a - t0) // 2, (b - t0) // 2
                        otc = ot_tiles[c_]
                        gsl = slice((r // 2) * 48 + k0, (r // 2) * 48 + k1)
                        for t in range(3):
                            ev = otc[:, t, a - CH * c_:b - CH * c_].rearrange(
                                "p (c two) -> p c two", two=2)[:, :, 0]
                            nc.vector.tensor_tensor(gout[:, t, gsl],
                                                    gout[:, t, gsl], ev,
                                                    op=OP.add)
                    r += 2
                emit_e.next_row = r
                if ch >= 2:
                    ot_tiles.pop(ch - 2, None)
            emit_e.next_row = 0

            stc, sta, ste = {}, {}, {}
            for ch in range(NCH):
                stc[ch] = emit_c1(ch)
                if ch >= 1:
                    sta[ch - 1] = emit_c2a(ch - 1, *stc.pop(ch - 1))
                if ch >= 2:
                    ste[ch - 2] = emit_c2b(ch - 2, *sta.pop(ch - 2))
                if ch >= 3:
                    emit_e(ch - 3, *ste.pop(ch - 3))
            sta[NCH - 1] = emit_c2a(NCH - 1, *stc.pop(NCH - 1))
            ste[NCH - 2] = emit_c2b(NCH - 2, *sta.pop(NCH - 2))
            emit_e(NCH - 3, *ste.pop(NCH - 3))
            ste[NCH - 1] = emit_c2b(NCH - 1, *sta.pop(NCH - 1))
            emit_e(NCH - 2, *ste.pop(NCH - 2))
            emit_e(NCH - 1, *ste.pop(NCH - 1))
            nc.sync.dma_start(
                OALL.rearrange("(t p) l -> p t l", p=128)[:, :, L:L + 1152],
                gout[:].bitcast(F32))

    nc.finalize()
    return nc


def _prep_core(x, sal_w, sal_b, attn_in_w, attn_in_b, attn_out_w, attn_out_b,
               w3, b3, w4, b4, core):
    n, hf = core // 2, core % 2
    hg, wg = n // 2, n % 2
    f = np.float32
    Xc = np.ascontiguousarray(
        x[n, :, 48 * hf:48 * hf + 48, :].reshape(D, L), dtype=f)
    PGc = np.ascontiguousarray(
        x[4, :, 48 * hg:48 * hg + 48, 48 * wg:48 * wg + 48].reshape(D, 2304)
    ).astype(mybir.dt.np(BF16))
    r0 = 48 * hg + 24 * hf
    GSc = np.ascontiguousarray(
        x[4, :, r0:r0 + 24, 48 * wg:48 * wg + 48].reshape(D, 1152), dtype=f)

    wq = attn_in_w[n, 0:D]
    wk = attn_in_w[n, D:2 * D]
    wv = attn_in_w[n, 2 * D:3 * D]
    bq = attn_in_b[n, 0:D]
    bk = attn_in_b[n, D:2 * D]
    bv = attn_in_b[n, 2 * D:3 * D]

    WQT = np.zeros((D, 512), f)
    WKT = np.zeros((D, 512), f)
    BQc = np.zeros((512, 1), f)
    BKc = np.zeros((512, 1), f)
    WVA = np.zeros((D + 1, 512), f)
    WOT = np.zeros((512, D), f)
    for h in range(NH):
        WQT[:, DP * h:DP * h + DH] = wq[DH * h:DH * (h + 1), :].T
        WKT[:, DP * h:DP * h + DH] = wk[DH * h:DH * (h + 1), :].T
        BQc[DP * h:DP * h + DH, 0] = bq[DH * h:DH * (h + 1)]
        BKc[DP * h:DP * h + DH, 0] = bk[DH * h:DH * (h + 1)]
        WVA[0:D, DP * h + 1:DP * h + 1 + DH] = wv[DH * h:DH * (h + 1), :].T
        WVA[D, DP * h + 1:DP * h + 1 + DH] = bv[DH * h:DH * (h + 1)]
        WVA[D, DP * h] = 1.0
        WOT[DP * h + 1:DP * h + 1 + DH, :] = attn_out_w[n, :, DH * h:DH * (h + 1)].T
    return {
        "X": Xc, "PG": PGc, "GS": GSc,
        "WQT": WQT, "WKT": WKT, "WVA": WVA,
        "WOT": np.ascontiguousarray(WOT).astype(mybir.dt.np(BF16)),
        "W3T": np.ascontiguousarray(w3.T, dtype=f),
        "W4T": np.ascontiguousarray(w4.T).astype(mybir.dt.np(BF16)),
        "SW": np.ascontiguousarray(sal_w.reshape(1, D).T, dtype=f),
        "ONESR": np.ones((1, 756), f),
        "ONESD": np.full((128, 1), 1.0 / D, f),
        "ONESDB": np.full((128, 1), 1.0 / D, mybir.dt.np(BF16)),
        "SEL9": _sel9(),
        "BQ": BQc, "BK": BKc,
        "BO": np.asarray(attn_out_b[n], f).reshape(D, 1),
        "B3": np.asarray(b3, f).reshape(768, 1),
        "B4": np.asarray(b4, f).reshape(D, 1),
        "G1": None, "BE1": None, "G2": None, "BE2": None,
        "SALB": np.asarray(sal_b, f).reshape(1, 1),
    }


def kernel(x, sal_w, sal_b, attn_in_w, attn_in_b, attn_out_w, attn_out_b,
           w3, b3, w4, b4, g1, be1, g2, be2):
    f = np.float32
    x = np.asarray(x, f)
    args = [np.asarray(a, f) for a in
            (sal_w, sal_b, attn_in_w, attn_in_b, attn_out_w, attn_out_b,
             w3, b3, w4, b4)]
    g1, be1, g2, be2 = (np.asarray(a, f) for a in (g1, be1, g2, be2))
    use_g1 = not (np.all(g1 == 1.0) and np.all(be1 == 0.0))
    use_g2 = not (np.all(g2 == 1.0) and np.all(be2 == 0.0))

    key = (use_g1, use_g2)
    if key not in _cache:
        _cache[key] = _build(use_g1, use_g2)
    nc = _cache[key]

    in_maps = []
    for core in range(8):
        m = _prep_core(x, *args, core)
        m["G1"] = g1.reshape(D, 1)
        m["BE1"] = be1.reshape(D, 1)
        m["G2"] = g2.reshape(D, 1)
        m["BE2"] = be2.reshape(D, 1)
        in_maps.append(m)

    res = run_bass_kernel_spmd(nc, in_maps, list(range(8)))

    out = np.empty((5, D, HW, HW), f)
    for core in range(8):
        n, hf = core // 2, core % 2
        hg, wg = n // 2, n % 2
        oall = res.results[core]["OALL"]
        osrc = oall[:, 0:L].reshape(D, 48, 96)
        out[n, :, 48 * hf:48 * hf + 48, :] = osrc
        oglb = oall[:, L:L + 1152].reshape(D, 24, 48)
        r0 = 48 * hg + 24 * hf
        out[4, :, r0:r0 + 24, 48 * wg:48 * wg + 48] = oglb
    return out



# revision 37
# speedup vs baseline: 143.5047x; 1.0316x over previous
"""Trainium2 Bass kernel for the MCRM block (4 local patches + global branch).

Sharding: 8 cores = 4 patches x 2 token-halves. Each core runs the full
attention+FFN pipeline for 4608 tokens of one patch in feature-major layout
(channels on partitions). The small global-branch work (saliency map, pooled
KV, glb output slice) is computed per-core on its slice.

Structure (per core, ~600 us HW time):
- setup: saliency map; 2x2 KV pooling with the row-pair stage folded into
  the PG load via an accumulating DMA; K/V projections. Chunk 0's X load,
  mask and Q projection are emitted early so the PE isn't idle behind the
  DVE pooling chain.
- phase A (attention), software-pipelined over 9 chunks of 512 tokens:
  per iteration the PE gets Q_ch | WO+stats(ch-2) | scores+AV_ch | sts(ch-2)
  so the softmax-normalize chain (reciprocal_approx_fast + gpsimd broadcast
  + multiply) of chunk ch resolves under the next chunk's matmuls and never
  stalls the PE. AV accumulates head PAIRS into one PSUM bank (tile_position
  column halves); the out-projection then contracts 128 rows per matmul.
  LN1 row stats stream to 16-partition tiles; one batched sqrt+recip at the
  end keeps the ACT Exp table resident all phase.
- phase C+E fused, software-pipelined: LN1 apply + W3+gelu (ch) | W4 +
  residual + LN2 stats (ch-1, per-chunk sqrt/fast-recip) | LN2 apply +
  per-chunk DRAM store (ch-2). The glb branch accumulates per-chunk into
  gout (straddling rows split into segments), so there is no full-L second
  buffer.

Each core emits ONE merged DRAM output OALL[D, L+1152] (src tokens in
columns [0:L], the glb slice in [L:L+1152]); extra output arrays cost a
full client round-trip each on the axon-tunneled execution path.
"""
import sys
sys.path.insert(0, '/opt/trn_rl_repo')
import numpy as np

import concourse.bacc as bacc
import concourse.mybir as mybir
import concourse.tile as tile
from concourse.bass_utils import run_bass_kernel_spmd

F32 = mybir.dt.float32
F32R = mybir.dt.float32r
BF16 = mybir.dt.bfloat16
AF = mybir.ActivationFunctionType
OP = mybir.AluOpType

D = 384          # d_model
NH = 8           # heads
DH = 48          # head dim
DP = 64          # padded head dim
HW = 96
L = 4608         # tokens per core (half patch)
CH = 512         # token chunk
NCH = L // CH    # 9
S = 756          # kv tokens (576+144+36)
NST = 6          # s-tiles of 126
ST = 126
SCALE = 1.0 / np.sqrt(48.0)
EPS = 1e-5

_cache = {}


def _sel9():
    s = np.zeros((16, NCH * 128), np.float32)
    for ch in range(NCH):
        s[ch, 128 * ch:128 * (ch + 1)] = 1.0
    return s


def _build(use_g1, use_g2):
    nc = bacc.Bacc(target_bir_lowering=False, debug=False)

    def dparam(name, shape, dt=F32R):
        return nc.declare_dram_parameter(name, list(shape), dt, isOutput=False)

    X = dparam("X", (D, L))
    PG = dparam("PG", (D, 2304), BF16)
    GS = dparam("GS", (D, 1152))
    WQT = dparam("WQT", (D, 512))
    WKT = dparam("WKT", (D, 512))
    WVA = dparam("WVA", (D + 1, 512))
    WOT = dparam("WOT", (512, D), BF16)
    W3T = dparam("W3T", (D, 768))
    W4T = dparam("W4T", (768, D), BF16)
    SW = dparam("SW", (D, 1))
    ONESR = dparam("ONESR", (1, 756))
    ONESD = dparam("ONESD", (128, 1))          # value 1/384
    ONESDB = dparam("ONESDB", (128, 1), BF16)  # value 1/384
    SEL9 = dparam("SEL9", (16, NCH * 128))     # one-hot row selectors
    BQ = dparam("BQ", (512, 1), F32)
    BK = dparam("BK", (512, 1), F32)
    BO = dparam("BO", (D, 1), F32)
    B3 = dparam("B3", (768, 1), F32)
    B4 = dparam("B4", (D, 1), F32)
    G1 = dparam("G1", (D, 1), F32)
    BE1 = dparam("BE1", (D, 1), F32)
    G2 = dparam("G2", (D, 1), F32)
    BE2 = dparam("BE2", (D, 1), F32)
    SALB = dparam("SALB", (1, 1), F32)
    # single merged output: columns [0:L] = src slice, [L:L+1152] = glb slice
    # (one output array per call: each extra output costs a full client
    # round-trip on the remote execution path)
    OALL = nc.declare_dram_parameter("OALL", [D, L + 1152], F32, isOutput=True)

    NP = 16  # partition count for stats tiles (>= NCH)

    with tile.TileContext(nc) as tc:
      with tc.tile_pool(name="const", bufs=1) as cp:
        onesd = cp.tile([128, 1], F32R)
        nc.sync.dma_start(onesd[:], ONESD[:])
        onesdb = cp.tile([128, 1], BF16)
        nc.sync.dma_start(onesdb[:], ONESDB[:])
        sel9 = cp.tile([NP, NCH * 128], F32R)
        nc.sync.dma_start(sel9[:], SEL9[:])
        salb = cp.tile([1, 1], F32)
        nc.sync.dma_start(salb[:], SALB[:])
        eps_t = cp.tile([NP, 1], F32)
        nc.vector.memset(eps_t[:], float(EPS))

        def col(par, n, tg):
            t = cp.tile([128, n // 128, 1], F32, tag=tg)
            nc.sync.dma_start(t[:], par.rearrange("(t p) o -> p t o", p=128))
            return t
        bq_c = col(BQ, 512, "bqc")
        bo_c = col(BO, D, "boc")
        b3_c = col(B3, 768, "b3c")
        b4_c = col(B4, D, "b4c")
        g1_c = col(G1, D, "g1c")
        be1_c = col(BE1, D, "be1c")
        g2_c = col(G2, D, "g2c")
        be2_c = col(BE2, D, "be2c")

        # per-chunk LN stats, one row per chunk (partition = chunk index)
        m19 = cp.tile([NP, CH], F32R)
        rs19 = cp.tile([NP, CH], F32R)
        m29 = cp.tile([NP, CH], F32R)
        rs29 = cp.tile([NP, CH], F32R)
        tamup = cp.tile([1, L], F32)
        for _s in (m19, rs19, m29, rs29):
            nc.vector.memset(_s[:].bitcast(F32), 0.0)

        with tc.tile_pool(name="bigA", bufs=1) as bpa:
          src = bpa.tile([128, 3, L], F32R, tag="bigA")
          with tc.tile_pool(name="apool", bufs=1) as apl:
            wq = apl.tile([128, 3, 512], F32R)
            nc.sync.dma_start(wq[:], WQT.rearrange("(t p) m -> p t m", p=128))
            wo = apl.tile([128, 4, D], BF16)
            nc.sync.dma_start(wo[:], WOT.rearrange("(h p) m -> p h m", p=128))
            k_sb = apl.tile([128, 4, S], F32R)
            v_sb = apl.tile([126, 6, 512], BF16)

            # ------------- setup: tam, pooling, K/V proj -------------
            with tc.tile_pool(name="setup", bufs=1) as spl, \
                 tc.tile_pool(name="setps", bufs=2, space="PSUM") as sps:
                wk = spl.tile([128, 3, 512], F32R)
                nc.sync.dma_start(wk[:], WKT.rearrange("(t p) m -> p t m", p=128))
                wv = spl.tile([128, 3, 512], F32R)
                nc.sync.dma_start(wv[:], WVA[0:D, :].rearrange("(t p) m -> p t m", p=128))
                wv1 = spl.tile([1, 512], F32R)
                nc.sync.dma_start(wv1[:], WVA[D:D + 1, :])
                sw = spl.tile([128, 3, 1], F32R)
                nc.sync.dma_start(sw[:], SW.rearrange("(t p) o -> p t o", p=128))
                onesr = spl.tile([1, 756], F32R)
                nc.sync.dma_start(onesr[:], ONESR[:])
                bk_c = spl.tile([128, 4, 1], F32)
                nc.sync.dma_start(bk_c[:], BK.rearrange("(t p) o -> p t o", p=128))
                gs = spl.tile([128, 3, 1152], F32R)
                nc.sync.dma_start(gs[:], GS.rearrange("(t p) m -> p t m", p=128))
                pg = spl.tile([128, 3, 2304], BF16)
                nc.sync.dma_start(pg[:], PG.rearrange("(t p) m -> p t m", p=128))

                uview = tamup[:].rearrange("o (r a c b) -> o r a c b", r=24, a=2, b=2)
                for nchk in range(3):
                    tp = sps.tile([1, 384], F32, tag="tamps")
                    for t in range(3):
                        nc.tensor.matmul(tp[:], sw[:, t, :],
                                         gs[:, t, 384 * nchk:384 * (nchk + 1)],
                                         start=(t == 0), stop=(t == 2))
                    tamc = spl.tile([1, 384], F32, tag="tamc", bufs=2,
                                    name=f"tamc_{nchk}")
                    nc.scalar.activation(tamc[:], tp[:],
                                         AF.Sigmoid, bias=salb[:, 0:1], scale=1.0)
                    tcv = tamc[:].rearrange("o (r c) -> o r c", r=8)
                    for a in range(2):
                        for b in range(2):
                            nc.vector.tensor_copy(
                                uview[:, 8 * nchk:8 * (nchk + 1), a, :, b], tcv[:])

                kv = spl.tile([128, 3, S], F32R)
                for t in range(3):
                    pgv = pg[:, t, :].rearrange("p (r a c) -> p r a c", r=24, a=2)
                    h1 = spl.tile([128, 24, 48], F32, tag="poolh")
                    nc.vector.tensor_tensor(h1[:], pgv[:, :, 0, :], pgv[:, :, 1, :],
                                            op=OP.add)
                    h1v = h1[:].rearrange("p r (c d) -> p r c d", d=2)
                    p4 = spl.tile([128, 24, 24], F32, tag="poolp4")
                    nc.vector.tensor_tensor(p4[:], h1v[:, :, :, 0], h1v[:, :, :, 1],
                                            op=OP.add)
                    nc.vector.tensor_scalar(kv[:, t, 0:576].rearrange(
                        "p (r c) -> p r c", r=24), p4[:], 0.25, None, op0=OP.mult)
                    k4 = kv[:, t, 0:576].rearrange("p (r a c) -> p r a c", r=12, a=2)
                    h2 = spl.tile([128, 12, 24], F32, tag="poolh2")
                    nc.vector.tensor_tensor(h2[:], k4[:, :, 0, :], k4[:, :, 1, :],
                                            op=OP.add)
                    h2v = h2[:].rearrange("p r (c d) -> p r c d", d=2)
                    p8 = spl.tile([128, 12, 12], F32, tag="poolp8")
                    nc.vector.tensor_tensor(p8[:], h2v[:, :, :, 0], h2v[:, :, :, 1],
                                            op=OP.add)
                    nc.vector.tensor_scalar(kv[:, t, 576:720].rearrange(
                        "p (r c) -> p r c", r=12), p8[:], 0.25, None, op0=OP.mult)
                    k8 = kv[:, t, 576:720].rearrange("p (r a c) -> p r a c", r=6, a=2)
                    h3 = spl.tile([128, 6, 12], F32, tag="poolh3")
                    nc.vector.tensor_tensor(h3[:], k8[:, :, 0, :], k8[:, :, 1, :],
                                            op=OP.add)
                    h3v = h3[:].rearrange("p r (c d) -> p r c d", d=2)
                    p16 = spl.tile([128, 6, 6], F32, tag="poolp16")
                    nc.vector.tensor_tensor(p16[:], h3v[:, :, :, 0], h3v[:, :, :, 1],
                                            op=OP.add)
                    nc.vector.tensor_scalar(kv[:, t, 720:756].rearrange(
                        "p (r c) -> p r c", r=6), p16[:], 0.25, None, op0=OP.mult)

                for mt in range(4):
                    for nchk in range(2):
                        nsl = slice(378 * nchk, 378 * (nchk + 1))
                        kp = sps.tile([128, 378], F32, tag="kvps")
                        for t in range(3):
                            nc.tensor.matmul(kp[:], wk[:, t, 128 * mt:128 * (mt + 1)],
                                             kv[:, t, nsl],
                                             start=(t == 0), stop=(t == 2))
                        nc.vector.tensor_scalar(k_sb[:, mt, nsl], kp[:],
                                                bk_c[:, mt, :], None, op0=OP.add)
                for st in range(NST):
                    ssl = slice(ST * st, ST * (st + 1))
                    vp = sps.tile([126, 512], F32, tag="kvps")
                    for t in range(3):
                        nc.tensor.matmul(vp[:], kv[:, t, ssl], wv[:, t, :],
                                         start=(t == 0), stop=False)
                    nc.tensor.matmul(vp[:], onesr[:, ssl], wv1[:],
                                     start=False, stop=True)
                    nc.vector.tensor_copy(v_sb[:, st, :], vp[:])

            # ------------- phase A: attention, software-pipelined -------------
            # Per iteration ch the engine queues get:
            #   PE : Q_ch | WO+stats(ch-2) | scores+AV_ch | sts(ch-2)
            #   DVE: mask_{ch+1} qbias_ch residual/sq_{ch-2} norm_ch stats_{ch-2}
            # so the softmax-normalize chain (recip+bcast+mult) of chunk ch
            # resolves during the next chunk's scores/AV and never stalls PE.
            with tc.tile_pool(name="wka", bufs=4) as wa, \
                 tc.tile_pool(name="tbp", bufs=2) as tbp, \
                 tc.tile_pool(name="qpool", bufs=4) as qpl, \
                 tc.tile_pool(name="epool", bufs=6) as ep, \
                 tc.tile_pool(name="aow", bufs=2) as aw, \
                 tc.tile_pool(name="aonp", bufs=9) as awn, \
                 tc.tile_pool(name="sqp", bufs=2) as sqp, \
                 tc.tile_pool(name="bigps", bufs=2, space="PSUM") as bps, \
                 tc.tile_pool(name="avps", bufs=3, space="PSUM") as avp:

                def emit_xdma(ch):
                    csl = slice(CH * ch, CH * (ch + 1))
                    xm = wa.tile([128, 3, CH], F32R, tag="xc")
                    nc.sync.dma_start(
                        xm[:], X.rearrange("(t p) l -> p t l", p=128)[:, :, csl])
                    return xm

                def emit_mask(ch, xm):
                    csl = slice(CH * ch, CH * (ch + 1))
                    tb = tbp.tile([128, CH], F32, tag="tb")
                    nc.gpsimd.partition_broadcast(tb[:], tamup[:, csl])
                    for t in range(3):
                        nc.vector.tensor_tensor(xm[:, t, :], xm[:, t, :], tb[:],
                                                op=OP.mult)

                def emit_q(ch, xm):
                    q = []
                    for mt in range(4):
                        qp = bps.tile([128, 2 * CH], F32, tag="ps")
                        for t in range(3):
                            nc.tensor.matmul(qp[:, 0:CH],
                                             wq[:, t, 128 * mt:128 * (mt + 1)],
                                             xm[:, t, :],
                                             start=(t == 0), stop=(t == 2))
                        qt = qpl.tile([128, CH], F32R, tag="q")
                        nc.vector.tensor_scalar(qt[:], qp[:, 0:CH],
                                                bq_c[:, mt, :], None, op0=OP.add)
                        q.append(qt)
                    return q

                def emit_attn(ch, q):
                    aon = []
                    for mt in range(4):          # head pair (heads 2mt, 2mt+1)
                        e_t = []
                        for st in range(NST):
                            ssl = slice(ST * st, ST * (st + 1))
                            sp = bps.tile([128, 2 * CH], F32, tag="ps")
                            nc.tensor.matmul(
                                sp[0:126, 0:CH],
                                k_sb[0:64, mt, ssl], q[mt][0:64, :],
                                start=True, stop=True, tile_position=(0, 0))
                            nc.tensor.matmul(
                                sp[0:126, CH:2 * CH],
                                k_sb[64:128, mt, ssl], q[mt][64:128, :],
                                start=True, stop=True, tile_position=(64, 0))
                            et = ep.tile([126, 2 * CH], BF16, tag="e")
                            nc.scalar.activation(et[:], sp[0:126, :],
                                                 AF.Exp, scale=float(SCALE))
                            e_t.append(et)
                        # both heads of the pair accumulate into one PSUM bank:
                        # rows 0:64 head 2mt, rows 64:128 head 2mt+1
                        ap_ = avp.tile([128, CH], F32, tag="av")
                        for st in range(NST):
                            nc.tensor.matmul(
                                ap_[0:64, :],
                                v_sb[:, st, 128 * mt:128 * mt + 64],
                                e_t[st][:, 0:CH],
                                start=(st == 0), stop=(st == NST - 1),
                                tile_position=(0, 0))
                            nc.tensor.matmul(
                                ap_[64:128, :],
                                v_sb[:, st, 128 * mt + 64:128 * (mt + 1)],
                                e_t[st][:, CH:2 * CH],
                                start=(st == 0), stop=(st == NST - 1),
                                tile_position=(0, 64))
                        rc0 = aw.tile([1, CH], F32, tag="rowtmp")
                        nc.vector.reciprocal_approx_fast(rc0[:], ap_[0:1, :])
                        rc1 = aw.tile([1, CH], F32, tag="rowtmp")
                        nc.vector.reciprocal_approx_fast(rc1[:], ap_[64:65, :])
                        # partition_broadcast only lands correctly on ranges
                        # based at partition 0 — broadcast each denominator to
                        # a full tile and multiply the matching lane halves
                        bca = aw.tile([128, CH], F32, tag="bc")
                        nc.gpsimd.partition_broadcast(bca[:], rc0[:])
                        bcb = aw.tile([128, CH], F32, tag="bc")
                        nc.gpsimd.partition_broadcast(bcb[:], rc1[:])
                        an = awn.tile([128, CH], BF16, tag="aon")
                        nc.vector.tensor_tensor(an[0:64, :], ap_[0:64, :],
                                                bca[0:64, :], op=OP.mult)
                        nc.vector.tensor_tensor(an[64:128, :], ap_[64:128, :],
                                                bcb[64:128, :], op=OP.mult)
                        aon.append(an)
                    return aon

                def emit_b1(ch, xm, aon):
                    # out-projection (contraction 128 = head pair) + residual
                    csl = slice(CH * ch, CH * (ch + 1))
                    for t in range(3):
                        op_ = bps.tile([128, 2 * CH], F32, tag="ps")
                        for mt in range(4):
                            nc.tensor.matmul(op_[:, 0:CH],
                                             wo[:, mt, 128 * t:128 * (t + 1)],
                                             aon[mt][:],
                                             start=(mt == 0), stop=(mt == 3))
                        nc.vector.scalar_tensor_tensor(
                            src[:, t, csl], op_[:, 0:CH], bo_c[:, t, :],
                            xm[:, t, :], op0=OP.add, op1=OP.add)
                    stm = bps.tile([128, 2 * CH], F32, tag="ps")
                    for t in range(3):
                        nc.tensor.matmul(stm[0:1, 0:CH], onesd[:], src[:, t, csl],
                                         start=(t == 0), stop=(t == 2))
                    mrow = aw.tile([1, CH], F32R, tag="mrow", bufs=2)
                    nc.vector.tensor_copy(mrow[:], stm[0:1, 0:CH])
                    nc.sync.dma_start(m19[ch:ch + 1, :], mrow[:])
                    sq = []
                    for t in range(3):
                        sqt = sqp.tile([128, CH], F32R, tag="sq", bufs=4)
                        nc.vector.tensor_tensor(sqt[:], src[:, t, csl],
                                                src[:, t, csl], op=OP.mult)
                        sq.append(sqt)
                    return sq

                def emit_b2(ch, sq):
                    sts = bps.tile([128, 2 * CH], F32, tag="ps")
                    for t in range(3):
                        nc.tensor.matmul(sts[0:1, 0:CH], onesd[:], sq[t][:],
                                         start=(t == 0), stop=(t == 2))
                    vrow = aw.tile([1, CH], F32R, tag="mrow", bufs=2)
                    nc.vector.tensor_copy(vrow[:], sts[0:1, 0:CH])
                    nc.gpsimd.dma_start(rs19[ch:ch + 1, :], vrow[:])

                st_xm = {0: emit_xdma(0), 1: emit_xdma(1)}
                emit_mask(0, st_xm[0])
                st_aon, st_b = {}, {}
                for ch in range(NCH):
                    if ch + 2 < NCH:
                        st_xm[ch + 2] = emit_xdma(ch + 2)
                    if ch + 1 < NCH:
                        emit_mask(ch + 1, st_xm[ch + 1])
                    q = emit_q(ch, st_xm[ch])
                    if ch >= 2:
                        st_b[ch - 2] = emit_b1(ch - 2, st_xm[ch - 2],
                                               st_aon.pop(ch - 2))
                        del st_xm[ch - 2]
                    st_aon[ch] = emit_attn(ch, q)
                    if ch >= 3:
                        emit_b2(ch - 3, st_b.pop(ch - 3))
                st_b[NCH - 2] = emit_b1(NCH - 2, st_xm.pop(NCH - 2),
                                        st_aon.pop(NCH - 2))
                emit_b2(NCH - 3, st_b.pop(NCH - 3))
                st_b[NCH - 1] = emit_b1(NCH - 1, st_xm.pop(NCH - 1),
                                        st_aon.pop(NCH - 1))
                emit_b2(NCH - 2, st_b.pop(NCH - 2))
                emit_b2(NCH - 1, st_b.pop(NCH - 1))

                # batched var = E[x^2] - mean^2 then 1/sqrt(var+eps),
                # all 9 chunks in one 16-wide pass
                msq = aw.tile([NP, CH], F32, tag="svall")
                nc.vector.tensor_tensor(msq[:], m19[:], m19[:], op=OP.mult)
                nc.vector.tensor_tensor(msq[:], rs19[:].bitcast(F32), msq[:],
                                        op=OP.subtract)
                sva = aw.tile([NP, CH], F32, tag="svall")
                nc.scalar.activation(sva[:], msq[:], AF.Sqrt,
                                     bias=eps_t[:, 0:1], scale=1.0)
                svb = aw.tile([NP, CH], F32, tag="svall")
                nc.vector.reciprocal_approx_fast(svb[:], sva[:])
                nc.vector.tensor_copy(rs19[:], svb[:])

          # ------- phase C+E fused: LN1+FFN+LN2+store, software-pipelined -------
          # Per iteration: PE gets mb/rb+W3(ch) | W4+stats2(ch-1); the LN2 of
          # chunk ch-2 (per-chunk sqrt/recip + gpsimd broadcasts + DVE apply)
          # resolves under the next chunks' matmuls. Output slices stream to
          # DRAM per chunk; the glb branch accumulates per-chunk into gout.
          with tc.tile_pool(name="cpool", bufs=1) as cpl, \
               tc.tile_pool(name="wkc", bufs=2) as wc, \
               tc.tile_pool(name="srmurow", bufs=3) as srp, \
               tc.tile_pool(name="cps", bufs=6, space="PSUM") as cps, \
               tc.tile_pool(name="cbc", bufs=2, space="PSUM") as cbc:
            w3 = cpl.tile([128, 3, 768], F32R)
            nc.sync.dma_start(w3[:], W3T.rearrange("(t p) m -> p t m", p=128))
            w4 = cpl.tile([128, 6, D], BF16)
            nc.sync.dma_start(w4[:], W4T.rearrange("(t p) m -> p t m", p=128))
            gout = cpl.tile([128, 3, 1152], F32R)
            nc.sync.dma_start(gout[:], GS.rearrange("(t p) m -> p t m", p=128))

            def emit_c1(ch):
                # LN1 apply + W3 + gelu
                csl = slice(CH * ch, CH * (ch + 1))
                ssel = sel9[:, 128 * ch:128 * (ch + 1)]
                mb = cbc.tile([128, CH], F32, tag="cbc")
                nc.tensor.matmul(mb[:], ssel, m19[:], start=True, stop=True)
                rb = cbc.tile([128, CH], F32, tag="cbc")
                nc.tensor.matmul(rb[:], ssel, rs19[:], start=True, stop=True)
                sln = wc.tile([128, 3, CH], F32R, tag="sln")
                for t in range(3):
                    ctr = wc.tile([128, CH], F32, tag="ctr")
                    nc.vector.tensor_tensor(ctr[:], src[:, t, csl], mb[:],
                                            op=OP.subtract)
                    nc.vector.tensor_tensor(sln[:, t, :], ctr[:], rb[:],
                                            op=OP.mult)
                    if use_g1:
                        nc.vector.tensor_scalar(sln[:, t, :], sln[:, t, :],
                                                g1_c[:, t, :], be1_c[:, t, :],
                                                op0=OP.mult, op1=OP.add)
                hid = wc.tile([128, 6, CH], BF16, tag="hid")
                for mt in range(6):
                    hp = cps.tile([128, CH], F32, tag="cp")
                    for t in range(3):
                        nc.tensor.matmul(hp[:], w3[:, t, 128 * mt:128 * (mt + 1)],
                                         sln[:, t, :],
                                         start=(t == 0), stop=(t == 2))
                    nc.scalar.activation(hid[:, mt, :], hp[:], AF.Gelu,
                                         bias=b3_c[:, mt, :], scale=1.0)
                return sln, hid

            def emit_c2a(ch, sln, hid):
                # W4 + residual + LN2 mean; squares go to gpsimd
                s2 = srp.tile([128, 3, CH], BF16, tag="s2", bufs=3)
                for t in range(3):
                    fp = cps.tile([128, CH], F32, tag="cp")
                    for kt in range(6):
                        nc.tensor.matmul(fp[:], w4[:, kt, 128 * t:128 * (t + 1)],
                                         hid[:, kt, :],
                                         start=(kt == 0), stop=(kt == 5))
                    nc.vector.scalar_tensor_tensor(
                        s2[:, t, :], fp[:], b4_c[:, t, :], sln[:, t, :],
                        op0=OP.add, op1=OP.add)
                stm2 = cps.tile([128, CH], F32, tag="cp")
                for t in range(3):
                    nc.tensor.matmul(stm2[0:1, :], onesdb[:], s2[:, t, :],
                                     start=(t == 0), stop=(t == 2))
                mrow = srp.tile([1, CH], F32, tag="mrow")
                nc.vector.tensor_copy(mrow[:], stm2[0:1, :])
                sq2l = []
                for t in range(3):
                    sq2 = wc.tile([128, CH], BF16, tag="sq2", bufs=6)
                    nc.vector.tensor_tensor(sq2[:], s2[:, t, :], s2[:, t, :],
                                            op=OP.mult)
                    sq2l.append(sq2)
                return s2, mrow, sq2l

            def emit_c2b(ch, s2, mrow, sq2l):
                sts2 = cps.tile([128, CH], F32, tag="cp")
                for t in range(3):
                    nc.tensor.matmul(sts2[0:1, :], onesdb[:], sq2l[t][:],
                                     start=(t == 0), stop=(t == 2))
                vrow = srp.tile([1, CH], F32, tag="vrow", bufs=2)
                nc.vector.tensor_tensor(vrow[:], mrow[:], mrow[:], op=OP.mult)
                nc.vector.tensor_tensor(vrow[:], sts2[0:1, :], vrow[:],
                                        op=OP.subtract)
                svr = srp.tile([1, CH], F32, tag="vrow", bufs=2)
                nc.scalar.activation(svr[:], vrow[:], AF.Sqrt,
                                     bias=eps_t[0:1, 0:1], scale=1.0)
                rrow = srp.tile([1, CH], F32, tag="rrow")
                nc.vector.reciprocal_approx_fast(rrow[:], svr[:])
                return s2, mrow, rrow

            ot_tiles = {}

            def emit_e(ch, s2, mrow, rrow):
                # LN2 apply via gpsimd broadcasts + store the chunk
                csl = slice(CH * ch, CH * (ch + 1))
                mb2 = srp.tile([128, CH], F32, tag="bc2")
                nc.gpsimd.partition_broadcast(mb2[:], mrow[:])
                rb2 = srp.tile([128, CH], F32, tag="bc2")
                nc.gpsimd.partition_broadcast(rb2[:], rrow[:])
                ot = srp.tile([128, 3, CH], F32, tag="ot")
                ot_tiles[ch] = ot
                for t in range(3):
                    ctr2 = wc.tile([128, CH], F32, tag="ctr2")
                    nc.vector.tensor_tensor(ctr2[:], s2[:, t, :], mb2[:],
                                            op=OP.subtract)
                    nc.vector.tensor_tensor(ot[:, t, :], ctr2[:], rb2[:],
                                            op=OP.mult)
                    if use_g2:
                        nc.vector.tensor_scalar(ot[:, t, :], ot[:, t, :],
                                                g2_c[:, t, :], be2_c[:, t, :],
                                                op0=OP.mult, op1=OP.add)
                nc.sync.dma_start(
                    OALL.rearrange("(t p) l -> p t l", p=128)[:, :, csl],
                    ot[:])
                # glb accumulation: even rows fully stored by now, in-place
                # add their even columns into gout (rows may straddle chunks)
                lim = CH * (ch + 1)
                r = emit_e.next_row
                while 96 * (r + 1) <= lim:
                    t0, t1 = 96 * r, 96 * (r + 1)
                    segs = []
                    cs = t0 // CH
                    if t1 - 1 >= CH * (cs + 1):
                        m = CH * (cs + 1)
                        segs = [(cs, t0, m), (cs + 1, m, t1)]
                    else:
                        segs = [(cs, t0, t1)]
                    for (c_, a, b) in segs:
                        k0, k1 = (a - t0) // 2, (b - t0) // 2
                        otc = ot_tiles[c_]
                        gsl = slice((r // 2) * 48 + k0, (r // 2) * 48 + k1)
                        for t in range(3):
                            ev = otc[:, t, a - CH * c_:b - CH * c_].rearrange(
                                "p (c two) -> p c two", two=2)[:, :, 0]
                            nc.vector.tensor_tensor(gout[:, t, gsl],
                                                    gout[:, t, gsl], ev,
                                                    op=OP.add)
                    r += 2
                emit_e.next_row = r
                if ch >= 2:
                    ot_tiles.pop(ch - 2, None)
            emit_e.next_row = 0

            stc, sta, ste = {}, {}, {}
            for ch in range(NCH):
                stc[ch] = emit_c1(ch)
                if ch >= 1:
                    sta[ch - 1] = emit_c2a(ch - 1, *stc.pop(ch - 1))
                if ch >= 2:
                    ste[ch - 2] = emit_c2b(ch - 2, *sta.pop(ch - 2))
                if ch >= 3:
                    emit_e(ch - 3, *ste.pop(ch - 3))
            sta[NCH - 1] = emit_c2a(NCH - 1, *stc.pop(NCH - 1))
            ste[NCH - 2] = emit_c2b(NCH - 2, *sta.pop(NCH - 2))
            emit_e(NCH - 3, *ste.pop(NCH - 3))
            ste[NCH - 1] = emit_c2b(NCH - 1, *sta.pop(NCH - 1))
            emit_e(NCH - 2, *ste.pop(NCH - 2))
            emit_e(NCH - 1, *ste.pop(NCH - 1))
            nc.sync.dma_start(
                OALL.rearrange("(t p) l -> p t l", p=128)[:, :, L:L + 1152],
                gout[:].bitcast(F32))

    nc.finalize()
    return nc


def _prep_core(x, sal_w, sal_b, attn_in_w, attn_in_b, attn_out_w, attn_out_b,
               w3, b3, w4, b4, core):
    n, hf = core // 2, core % 2
    hg, wg = n // 2, n % 2
    f = np.float32
    Xc = np.ascontiguousarray(
        x[n, :, 48 * hf:48 * hf + 48, :].reshape(D, L), dtype=f)
    PGc = np.ascontiguousarray(
        x[4, :, 48 * hg:48 * hg + 48, 48 * wg:48 * wg + 48].reshape(D, 2304)
    ).astype(mybir.dt.np(BF16))
    r0 = 48 * hg + 24 * hf
    GSc = np.ascontiguousarray(
        x[4, :, r0:r0 + 24, 48 * wg:48 * wg + 48].reshape(D, 1152), dtype=f)

    wq = attn_in_w[n, 0:D]
    wk = attn_in_w[n, D:2 * D]
    wv = attn_in_w[n, 2 * D:3 * D]
    bq = attn_in_b[n, 0:D]
    bk = attn_in_b[n, D:2 * D]
    bv = attn_in_b[n, 2 * D:3 * D]

    WQT = np.zeros((D, 512), f)
    WKT = np.zeros((D, 512), f)
    BQc = np.zeros((512, 1), f)
    BKc = np.zeros((512, 1), f)
    WVA = np.zeros((D + 1, 512), f)
    WOT = np.zeros((512, D), f)
    for h in range(NH):
        WQT[:, DP * h:DP * h + DH] = wq[DH * h:DH * (h + 1), :].T
        WKT[:, DP * h:DP * h + DH] = wk[DH * h:DH * (h + 1), :].T
        BQc[DP * h:DP * h + DH, 0] = bq[DH * h:DH * (h + 1)]
        BKc[DP * h:DP * h + DH, 0] = bk[DH * h:DH * (h + 1)]
        WVA[0:D, DP * h + 1:DP * h + 1 + DH] = wv[DH * h:DH * (h + 1), :].T
        WVA[D, DP * h + 1:DP * h + 1 + DH] = bv[DH * h:DH * (h + 1)]
        WVA[D, DP * h] = 1.0
        WOT[DP * h + 1:DP * h + 1 + DH, :] = attn_out_w[n, :, DH * h:DH * (h + 1)].T
    return {
        "X": Xc, "PG": PGc, "GS": GSc,
        "WQT": WQT, "WKT": WKT, "WVA": WVA,
        "WOT": np.ascontiguousarray(WOT).astype(mybir.dt.np(BF16)),
        "W3T": np.ascontiguousarray(w3.T, dtype=f),
        "W4T": np.ascontiguousarray(w4.T).astype(mybir.dt.np(BF16)),
        "SW": np.ascontiguousarray(sal_w.reshape(1, D).T, dtype=f),
        "ONESR": np.ones((1, 756), f),
        "ONESD": np.full((128, 1), 1.0 / D, f),
        "ONESDB": np.full((128, 1), 1.0 / D, mybir.dt.np(BF16)),
        "SEL9": _sel9(),
        "BQ": BQc, "BK": BKc,
        "BO": np.asarray(attn_out_b[n], f).reshape(D, 1),
        "B3": np.asarray(b3, f).reshape(768, 1),
        "B4": np.asarray(b4, f).reshape(D, 1),
        "G1": None, "BE1": None, "G2": None, "BE2": None,
        "SALB": np.asarray(sal_b, f).reshape(1, 1),
    }


def kernel(x, sal_w, sal_b, attn_in_w, attn_in_b, attn_out_w, attn_out_b,
           w3, b3, w4, b4, g1, be1, g2, be2):
    f = np.float32
    x = np.asarray(x, f)
    args = [np.asarray(a, f) for a in
            (sal_w, sal_b, attn_in_w, attn_in_b, attn_out_w, attn_out_b,
             w3, b3, w4, b4)]
    g1, be1, g2, be2 = (np.asarray(a, f) for a in (g1, be1, g2, be2))
    use_g1 = not (np.all(g1 == 1.0) and np.all(be1 == 0.0))
    use_g2 = not (np.all(g2 == 1.0) and np.all(be2 == 0.0))

    key = (use_g1, use_g2)
    if key not in _cache:
        _cache[key] = _build(use_g1, use_g2)
    nc = _cache[key]

    in_maps = []
    for core in range(8):
        m = _prep_core(x, *args, core)
        m["G1"] = g1.reshape(D, 1)
        m["BE1"] = be1.reshape(D, 1)
        m["G2"] = g2.reshape(D, 1)
        m["BE2"] = be2.reshape(D, 1)
        in_maps.append(m)

    res = run_bass_kernel_spmd(nc, in_maps, list(range(8)))

    out = np.empty((5, D, HW, HW), f)
    for core in range(8):
        n, hf = core // 2, core % 2
        hg, wg = n // 2, n % 2
        oall = res.results[core]["OALL"]
        osrc = oall[:, 0:L].reshape(D, 48, 96)
        out[n, :, 48 * hf:48 * hf + 48, :] = osrc
        oglb = oall[:, L:L + 1152].reshape(D, 24, 48)
        r0 = 48 * hg + 24 * hf
        out[4, :, r0:r0 + 24, 48 * wg:48 * wg + 48] = oglb
    return out



# revision 39
# speedup vs baseline: 143.7464x; 1.0017x over previous
"""Trainium2 Bass kernel for the MCRM block (4 local patches + global branch).

Sharding: 8 cores = 4 patches x 2 token-halves. Each core runs the full
attention+FFN pipeline for 4608 tokens of one patch in feature-major layout
(channels on partitions). The small global-branch work (saliency map, pooled
KV, glb output slice) is computed per-core on its slice.

Structure (per core, ~600 us HW time):
- setup: saliency map; 2x2 KV pooling with the row-pair stage folded into
  the PG load via an accumulating DMA; K/V projections. Chunk 0's X load,
  mask and Q projection are emitted early so the PE isn't idle behind the
  DVE pooling chain.
- phase A (attention), software-pipelined over 9 chunks of 512 tokens:
  per iteration the PE gets Q_ch | WO+stats(ch-2) | scores+AV_ch | sts(ch-2)
  so the softmax-normalize chain (reciprocal_approx_fast + gpsimd broadcast
  + multiply) of chunk ch resolves under the next chunk's matmuls and never
  stalls the PE. AV accumulates head PAIRS into one PSUM bank (tile_position
  column halves); the out-projection then contracts 128 rows per matmul.
  LN1 row stats stream to 16-partition tiles; one batched sqrt+recip at the
  end keeps the ACT Exp table resident all phase.
- phase C+E fused, software-pipelined: LN1 apply + W3+gelu (ch) | W4 +
  residual + LN2 stats (ch-1, per-chunk sqrt/fast-recip) | LN2 apply +
  per-chunk DRAM store (ch-2). The glb branch accumulates per-chunk into
  gout (straddling rows split into segments), so there is no full-L second
  buffer.

Each core emits ONE merged DRAM output OALL[D, L+1152] (src tokens in
columns [0:L], the glb slice in [L:L+1152]); extra output arrays cost a
full client round-trip each on the axon-tunneled execution path.
"""
import sys
sys.path.insert(0, '/opt/trn_rl_repo')
import numpy as np

import concourse.bacc as bacc
import concourse.mybir as mybir
import concourse.tile as tile
from concourse.bass_utils import run_bass_kernel_spmd

F32 = mybir.dt.float32
F32R = mybir.dt.float32r
BF16 = mybir.dt.bfloat16
AF = mybir.ActivationFunctionType
OP = mybir.AluOpType

D = 384          # d_model
NH = 8           # heads
DH = 48          # head dim
DP = 64          # padded head dim
HW = 96
L = 4608         # tokens per core (half patch)
CH = 512         # token chunk
NCH = L // CH    # 9
S = 756          # kv tokens (576+144+36)
NST = 6          # s-tiles of 126
ST = 126
SCALE = 1.0 / np.sqrt(48.0)
EPS = 1e-5

_cache = {}


def _sel9():
    s = np.zeros((16, NCH * 128), np.float32)
    for ch in range(NCH):
        s[ch, 128 * ch:128 * (ch + 1)] = 1.0
    return s


def _build(use_g1, use_g2):
    nc = bacc.Bacc(target_bir_lowering=False, debug=False)

    def dparam(name, shape, dt=F32R):
        return nc.declare_dram_parameter(name, list(shape), dt, isOutput=False)

    X = dparam("X", (D, L))
    PG = dparam("PG", (D, 2304), BF16)
    GS = dparam("GS", (D, 1152))
    WQT = dparam("WQT", (D, 512))
    WKT = dparam("WKT", (D, 512))
    WVA = dparam("WVA", (D + 1, 512))
    WOT = dparam("WOT", (512, D), BF16)
    W3T = dparam("W3T", (D, 768))
    W4T = dparam("W4T", (768, D), BF16)
    SW = dparam("SW", (D, 1))
    ONESR = dparam("ONESR", (1, 756))
    ONESD = dparam("ONESD", (128, 1))          # value 1/384
    ONESDB = dparam("ONESDB", (128, 1), BF16)  # value 1/384
    SEL9 = dparam("SEL9", (16, NCH * 128))     # one-hot row selectors
    BQ = dparam("BQ", (512, 1), F32)
    BK = dparam("BK", (512, 1), F32)
    BO = dparam("BO", (D, 1), F32)
    B3 = dparam("B3", (768, 1), F32)
    B4 = dparam("B4", (D, 1), F32)
    G1 = dparam("G1", (D, 1), F32)
    BE1 = dparam("BE1", (D, 1), F32)
    G2 = dparam("G2", (D, 1), F32)
    BE2 = dparam("BE2", (D, 1), F32)
    SALB = dparam("SALB", (1, 1), F32)
    # single merged output: columns [0:L] = src slice, [L:L+1152] = glb slice
    # (one output array per call: each extra output costs a full client
    # round-trip on the remote execution path)
    OALL = nc.declare_dram_parameter("OALL", [D, L + 1152], F32, isOutput=True)

    NP = 16  # partition count for stats tiles (>= NCH)

    with tile.TileContext(nc) as tc:
      with tc.tile_pool(name="const", bufs=1) as cp:
        onesd = cp.tile([128, 1], F32R)
        nc.sync.dma_start(onesd[:], ONESD[:])
        onesdb = cp.tile([128, 1], BF16)
        nc.sync.dma_start(onesdb[:], ONESDB[:])
        sel9 = cp.tile([NP, NCH * 128], F32R)
        nc.sync.dma_start(sel9[:], SEL9[:])
        salb = cp.tile([1, 1], F32)
        nc.sync.dma_start(salb[:], SALB[:])
        eps_t = cp.tile([NP, 1], F32)
        nc.vector.memset(eps_t[:], float(EPS))

        def col(par, n, tg):
            t = cp.tile([128, n // 128, 1], F32, tag=tg)
            nc.sync.dma_start(t[:], par.rearrange("(t p) o -> p t o", p=128))
            return t
        bq_c = col(BQ, 512, "bqc")
        bo_c = col(BO, D, "boc")
        b3_c = col(B3, 768, "b3c")
        b4_c = col(B4, D, "b4c")
        g1_c = col(G1, D, "g1c")
        be1_c = col(BE1, D, "be1c")
        g2_c = col(G2, D, "g2c")
        be2_c = col(BE2, D, "be2c")

        # per-chunk LN stats, one row per chunk (partition = chunk index)
        m19 = cp.tile([NP, CH], F32R)
        rs19 = cp.tile([NP, CH], F32R)
        rq19 = cp.tile([NP, CH], F32R)
        m29 = cp.tile([NP, CH], F32R)
        rs29 = cp.tile([NP, CH], F32R)
        tamup = cp.tile([1, L], F32)
        for _s in (m19, rs19, rq19, m29, rs29):
            nc.vector.memset(_s[:].bitcast(F32), 0.0)

        with tc.tile_pool(name="bigA", bufs=1) as bpa:
          src = bpa.tile([128, 3, L], F32R, tag="bigA")
          with tc.tile_pool(name="apool", bufs=1) as apl:
            wq = apl.tile([128, 3, 512], F32R)
            nc.sync.dma_start(wq[:], WQT.rearrange("(t p) m -> p t m", p=128))
            wo = apl.tile([128, 4, D], BF16)
            nc.sync.dma_start(wo[:], WOT.rearrange("(h p) m -> p h m", p=128))
            k_sb = apl.tile([128, 4, S], F32R)
            v_sb = apl.tile([126, 6, 512], BF16)

            # ------------- setup: tam, pooling, K/V proj -------------
            with tc.tile_pool(name="setup", bufs=1) as spl, \
                 tc.tile_pool(name="setps", bufs=2, space="PSUM") as sps:
                wk = spl.tile([128, 3, 512], F32R)
                nc.sync.dma_start(wk[:], WKT.rearrange("(t p) m -> p t m", p=128))
                wv = spl.tile([128, 3, 512], F32R)
                nc.sync.dma_start(wv[:], WVA[0:D, :].rearrange("(t p) m -> p t m", p=128))
                wv1 = spl.tile([1, 512], F32R)
                nc.sync.dma_start(wv1[:], WVA[D:D + 1, :])
                sw = spl.tile([128, 3, 1], F32R)
                nc.sync.dma_start(sw[:], SW.rearrange("(t p) o -> p t o", p=128))
                onesr = spl.tile([1, 756], F32R)
                nc.sync.dma_start(onesr[:], ONESR[:])
                bk_c = spl.tile([128, 4, 1], F32)
                nc.sync.dma_start(bk_c[:], BK.rearrange("(t p) o -> p t o", p=128))
                gs = spl.tile([128, 3, 1152], F32R)
                nc.sync.dma_start(gs[:], GS.rearrange("(t p) m -> p t m", p=128))
                pg = spl.tile([128, 3, 2304], BF16)
                nc.sync.dma_start(pg[:], PG.rearrange("(t p) m -> p t m", p=128))

                uview = tamup[:].rearrange("o (r a c b) -> o r a c b", r=24, a=2, b=2)
                for nchk in range(3):
                    tp = sps.tile([1, 384], F32, tag="tamps")
                    for t in range(3):
                        nc.tensor.matmul(tp[:], sw[:, t, :],
                                         gs[:, t, 384 * nchk:384 * (nchk + 1)],
                                         start=(t == 0), stop=(t == 2))
                    tamc = spl.tile([1, 384], F32, tag="tamc", bufs=2,
                                    name=f"tamc_{nchk}")
                    nc.scalar.activation(tamc[:], tp[:],
                                         AF.Sigmoid, bias=salb[:, 0:1], scale=1.0)
                    tcv = tamc[:].rearrange("o (r c) -> o r c", r=8)
                    for a in range(2):
                        for b in range(2):
                            nc.vector.tensor_copy(
                                uview[:, 8 * nchk:8 * (nchk + 1), a, :, b], tcv[:])

                kv = spl.tile([128, 3, S], F32R)
                for t in range(3):
                    pgv = pg[:, t, :].rearrange("p (r a c) -> p r a c", r=24, a=2)
                    h1 = spl.tile([128, 24, 48], F32, tag="poolh")
                    nc.vector.tensor_tensor(h1[:], pgv[:, :, 0, :], pgv[:, :, 1, :],
                                            op=OP.add)
                    h1v = h1[:].rearrange("p r (c d) -> p r c d", d=2)
                    p4 = spl.tile([128, 24, 24], F32, tag="poolp4")
                    nc.vector.tensor_tensor(p4[:], h1v[:, :, :, 0], h1v[:, :, :, 1],
                                            op=OP.add)
                    nc.vector.tensor_scalar(kv[:, t, 0:576].rearrange(
                        "p (r c) -> p r c", r=24), p4[:], 0.25, None, op0=OP.mult)
                    k4 = kv[:, t, 0:576].rearrange("p (r a c) -> p r a c", r=12, a=2)
                    h2 = spl.tile([128, 12, 24], F32, tag="poolh2")
                    nc.vector.tensor_tensor(h2[:], k4[:, :, 0, :], k4[:, :, 1, :],
                                            op=OP.add)
                    h2v = h2[:].rearrange("p r (c d) -> p r c d", d=2)
                    p8 = spl.tile([128, 12, 12], F32, tag="poolp8")
                    nc.vector.tensor_tensor(p8[:], h2v[:, :, :, 0], h2v[:, :, :, 1],
                                            op=OP.add)
                    nc.vector.tensor_scalar(kv[:, t, 576:720].rearrange(
                        "p (r c) -> p r c", r=12), p8[:], 0.25, None, op0=OP.mult)
                    k8 = kv[:, t, 576:720].rearrange("p (r a c) -> p r a c", r=6, a=2)
                    h3 = spl.tile([128, 6, 12], F32, tag="poolh3")
                    nc.vector.tensor_tensor(h3[:], k8[:, :, 0, :], k8[:, :, 1, :],
                                            op=OP.add)
                    h3v = h3[:].rearrange("p r (c d) -> p r c d", d=2)
                    p16 = spl.tile([128, 6, 6], F32, tag="poolp16")
                    nc.vector.tensor_tensor(p16[:], h3v[:, :, :, 0], h3v[:, :, :, 1],
                                            op=OP.add)
                    nc.vector.tensor_scalar(kv[:, t, 720:756].rearrange(
                        "p (r c) -> p r c", r=6), p16[:], 0.25, None, op0=OP.mult)

                for mt in range(4):
                    for nchk in range(2):
                        nsl = slice(378 * nchk, 378 * (nchk + 1))
                        kp = sps.tile([128, 378], F32, tag="kvps")
                        for t in range(3):
                            nc.tensor.matmul(kp[:], wk[:, t, 128 * mt:128 * (mt + 1)],
                                             kv[:, t, nsl],
                                             start=(t == 0), stop=(t == 2))
                        nc.vector.tensor_scalar(k_sb[:, mt, nsl], kp[:],
                                                bk_c[:, mt, :], None, op0=OP.add)
                for st in range(NST):
                    ssl = slice(ST * st, ST * (st + 1))
                    vp = sps.tile([126, 512], F32, tag="kvps")
                    for t in range(3):
                        nc.tensor.matmul(vp[:], kv[:, t, ssl], wv[:, t, :],
                                         start=(t == 0), stop=False)
                    nc.tensor.matmul(vp[:], onesr[:, ssl], wv1[:],
                                     start=False, stop=True)
                    nc.vector.tensor_copy(v_sb[:, st, :], vp[:])

            # ------------- phase A: attention, software-pipelined -------------
            # Per iteration ch the engine queues get:
            #   PE : Q_ch | WO+stats(ch-2) | scores+AV_ch | sts(ch-2)
            #   DVE: mask_{ch+1} qbias_ch residual/sq_{ch-2} norm_ch stats_{ch-2}
            # so the softmax-normalize chain (recip+bcast+mult) of chunk ch
            # resolves during the next chunk's scores/AV and never stalls PE.
            with tc.tile_pool(name="wka", bufs=4) as wa, \
                 tc.tile_pool(name="tbp", bufs=2) as tbp, \
                 tc.tile_pool(name="qpool", bufs=4) as qpl, \
                 tc.tile_pool(name="epool", bufs=6) as ep, \
                 tc.tile_pool(name="aow", bufs=2) as aw, \
                 tc.tile_pool(name="aonp", bufs=9) as awn, \
                 tc.tile_pool(name="sqp", bufs=2) as sqp, \
                 tc.tile_pool(name="bigps", bufs=2, space="PSUM") as bps, \
                 tc.tile_pool(name="avps", bufs=3, space="PSUM") as avp:

                def emit_xdma(ch):
                    csl = slice(CH * ch, CH * (ch + 1))
                    xm = wa.tile([128, 3, CH], F32R, tag="xc")
                    nc.sync.dma_start(
                        xm[:], X.rearrange("(t p) l -> p t l", p=128)[:, :, csl])
                    return xm

                def emit_mask(ch, xm):
                    csl = slice(CH * ch, CH * (ch + 1))
                    tb = tbp.tile([128, CH], F32, tag="tb")
                    nc.gpsimd.partition_broadcast(tb[:], tamup[:, csl])
                    for t in range(3):
                        nc.vector.tensor_tensor(xm[:, t, :], xm[:, t, :], tb[:],
                                                op=OP.mult)

                def emit_q(ch, xm):
                    q = []
                    for mt in range(4):
                        qp = bps.tile([128, 2 * CH], F32, tag="ps")
                        for t in range(3):
                            nc.tensor.matmul(qp[:, 0:CH],
                                             wq[:, t, 128 * mt:128 * (mt + 1)],
                                             xm[:, t, :],
                                             start=(t == 0), stop=(t == 2))
                        qt = qpl.tile([128, CH], F32R, tag="q")
                        nc.vector.tensor_scalar(qt[:], qp[:, 0:CH],
                                                bq_c[:, mt, :], None, op0=OP.add)
                        q.append(qt)
                    return q

                def emit_attn(ch, q):
                    aon = []
                    for mt in range(4):          # head pair (heads 2mt, 2mt+1)
                        e_t = []
                        for st in range(NST):
                            ssl = slice(ST * st, ST * (st + 1))
                            sp = bps.tile([128, 2 * CH], F32, tag="ps")
                            nc.tensor.matmul(
                                sp[0:126, 0:CH],
                                k_sb[0:64, mt, ssl], q[mt][0:64, :],
                                start=True, stop=True, tile_position=(0, 0))
                            nc.tensor.matmul(
                                sp[0:126, CH:2 * CH],
                                k_sb[64:128, mt, ssl], q[mt][64:128, :],
                                start=True, stop=True, tile_position=(64, 0))
                            et = ep.tile([126, 2 * CH], BF16, tag="e")
                            nc.scalar.activation(et[:], sp[0:126, :],
                                                 AF.Exp, scale=float(SCALE))
                            e_t.append(et)
                        # both heads of the pair accumulate into one PSUM bank:
                        # rows 0:64 head 2mt, rows 64:128 head 2mt+1
                        ap_ = avp.tile([128, CH], F32, tag="av")
                        for st in range(NST):
                            nc.tensor.matmul(
                                ap_[0:64, :],
                                v_sb[:, st, 128 * mt:128 * mt + 64],
                                e_t[st][:, 0:CH],
                                start=(st == 0), stop=(st == NST - 1),
                                tile_position=(0, 0))
                            nc.tensor.matmul(
                                ap_[64:128, :],
                                v_sb[:, st, 128 * mt + 64:128 * (mt + 1)],
                                e_t[st][:, CH:2 * CH],
                                start=(st == 0), stop=(st == NST - 1),
                                tile_position=(0, 64))
                        rc0 = aw.tile([1, CH], F32, tag="rowtmp")
                        nc.vector.reciprocal_approx_fast(rc0[:], ap_[0:1, :])
                        rc1 = aw.tile([1, CH], F32, tag="rowtmp")
                        nc.vector.reciprocal_approx_fast(rc1[:], ap_[64:65, :])
                        # partition_broadcast only lands correctly on ranges
                        # based at partition 0 — broadcast each denominator to
                        # a full tile and multiply the matching lane halves
                        bca = aw.tile([128, CH], F32, tag="bc")
                        nc.gpsimd.partition_broadcast(bca[:], rc0[:])
                        bcb = aw.tile([128, CH], F32, tag="bc")
                        nc.gpsimd.partition_broadcast(bcb[:], rc1[:])
                        an = awn.tile([128, CH], BF16, tag="aon")
                        nc.vector.tensor_tensor(an[0:64, :], ap_[0:64, :],
                                                bca[0:64, :], op=OP.mult)
                        nc.vector.tensor_tensor(an[64:128, :], ap_[64:128, :],
                                                bcb[64:128, :], op=OP.mult)
                        aon.append(an)
                    return aon

                def emit_b1(ch, xm, aon):
                    # out-projection (contraction 128 = head pair) + residual
                    csl = slice(CH * ch, CH * (ch + 1))
                    for t in range(3):
                        op_ = bps.tile([128, 2 * CH], F32, tag="ps")
                        for mt in range(4):
                            nc.tensor.matmul(op_[:, 0:CH],
                                             wo[:, mt, 128 * t:128 * (t + 1)],
                                             aon[mt][:],
                                             start=(mt == 0), stop=(mt == 3))
                        nc.vector.scalar_tensor_tensor(
                            src[:, t, csl], op_[:, 0:CH], bo_c[:, t, :],
                            xm[:, t, :], op0=OP.add, op1=OP.add)
                    stm = bps.tile([128, 2 * CH], F32, tag="ps")
                    for t in range(3):
                        nc.tensor.matmul(stm[0:1, 0:CH], onesd[:], src[:, t, csl],
                                         start=(t == 0), stop=(t == 2))
                    mrow = aw.tile([1, CH], F32R, tag="mrow", bufs=2)
                    nc.vector.tensor_copy(mrow[:], stm[0:1, 0:CH])
                    nc.sync.dma_start(m19[ch:ch + 1, :], mrow[:])
                    sq = []
                    for t in range(3):
                        sqt = sqp.tile([128, CH], F32R, tag="sq", bufs=4)
                        nc.vector.tensor_tensor(sqt[:], src[:, t, csl],
                                                src[:, t, csl], op=OP.mult)
                        sq.append(sqt)
                    return sq

                def emit_b2(ch, sq):
                    sts = bps.tile([128, 2 * CH], F32, tag="ps")
                    for t in range(3):
                        nc.tensor.matmul(sts[0:1, 0:CH], onesd[:], sq[t][:],
                                         start=(t == 0), stop=(t == 2))
                    vrow = aw.tile([1, CH], F32R, tag="mrow", bufs=2)
                    nc.vector.tensor_copy(vrow[:], sts[0:1, 0:CH])
                    nc.sync.dma_start(rs19[ch:ch + 1, :], vrow[:])

                st_xm = {0: emit_xdma(0), 1: emit_xdma(1)}
                emit_mask(0, st_xm[0])
                st_aon, st_b = {}, {}
                for ch in range(NCH):
                    if ch + 2 < NCH:
                        st_xm[ch + 2] = emit_xdma(ch + 2)
                    if ch + 1 < NCH:
                        emit_mask(ch + 1, st_xm[ch + 1])
                    q = emit_q(ch, st_xm[ch])
                    if ch >= 2:
                        st_b[ch - 2] = emit_b1(ch - 2, st_xm[ch - 2],
                                               st_aon.pop(ch - 2))
                        del st_xm[ch - 2]
                    st_aon[ch] = emit_attn(ch, q)
                    if ch >= 3:
                        emit_b2(ch - 3, st_b.pop(ch - 3))
                stats_batch(NCH - 3)
                st_b[NCH - 2] = emit_b1(NCH - 2, st_xm.pop(NCH - 2),
                                        st_aon.pop(NCH - 2))
                emit_b2(NCH - 3, st_b.pop(NCH - 3))
                st_b[NCH - 1] = emit_b1(NCH - 1, st_xm.pop(NCH - 1),
                                        st_aon.pop(NCH - 1))
                emit_b2(NCH - 2, st_b.pop(NCH - 2))
                emit_b2(NCH - 1, st_b.pop(NCH - 1))

                # var = E[x^2] - mean^2 then 1/sqrt(var+eps) -> rq19;
                # rows [0, r1). rs19 keeps raw E[x^2] so the full-width
                # second pass can recompute rows 0:6 harmlessly.
                def stats_batch(r1):
                    msq = aw.tile([NP, CH], F32, tag="svall")
                    nc.vector.tensor_tensor(msq[0:r1, :], m19[0:r1, :],
                                            m19[0:r1, :], op=OP.mult)
                    nc.vector.tensor_tensor(msq[0:r1, :],
                                            rs19[0:r1, :].bitcast(F32),
                                            msq[0:r1, :], op=OP.subtract)
                    sva = aw.tile([NP, CH], F32, tag="svall")
                    nc.scalar.activation(sva[0:r1, :], msq[0:r1, :], AF.Sqrt,
                                         bias=eps_t[0:r1, 0:1], scale=1.0)
                    svb = aw.tile([NP, CH], F32, tag="svall")
                    nc.vector.reciprocal_approx_fast(svb[0:r1, :],
                                                     sva[0:r1, :])
                    nc.vector.tensor_copy(rq19[0:r1, :], svb[0:r1, :])
                stats_batch(NP)

          # ------- phase C+E fused: LN1+FFN+LN2+store, software-pipelined -------
          # Per iteration: PE gets mb/rb+W3(ch) | W4+stats2(ch-1); the LN2 of
          # chunk ch-2 (per-chunk sqrt/recip + gpsimd broadcasts + DVE apply)
          # resolves under the next chunks' matmuls. Output slices stream to
          # DRAM per chunk; the glb branch accumulates per-chunk into gout.
          with tc.tile_pool(name="cpool", bufs=1) as cpl, \
               tc.tile_pool(name="wkc", bufs=2) as wc, \
               tc.tile_pool(name="srmurow", bufs=3) as srp, \
               tc.tile_pool(name="cps", bufs=6, space="PSUM") as cps, \
               tc.tile_pool(name="cbc", bufs=2, space="PSUM") as cbc:
            w3 = cpl.tile([128, 3, 768], F32R)
            nc.sync.dma_start(w3[:], W3T.rearrange("(t p) m -> p t m", p=128))
            w4 = cpl.tile([128, 6, D], BF16)
            nc.sync.dma_start(w4[:], W4T.rearrange("(t p) m -> p t m", p=128))
            gout = cpl.tile([128, 3, 1152], F32R)
            nc.sync.dma_start(gout[:], GS.rearrange("(t p) m -> p t m", p=128))

            def emit_c1(ch):
                # LN1 apply + W3 + gelu
                csl = slice(CH * ch, CH * (ch + 1))
                nr = NCH - 3 if ch < NCH - 3 else NP
                ssel = sel9[0:nr, 128 * ch:128 * (ch + 1)]
                mb = cbc.tile([128, CH], F32, tag="cbc")
                nc.tensor.matmul(mb[:], ssel, m19[0:nr, :],
                                 start=True, stop=True)
                rb = cbc.tile([128, CH], F32, tag="cbc")
                nc.tensor.matmul(rb[:], ssel, rq19[0:nr, :],
                                 start=True, stop=True)
                sln = wc.tile([128, 3, CH], F32R, tag="sln")
                for t in range(3):
                    ctr = wc.tile([128, CH], F32, tag="ctr")
                    nc.vector.tensor_tensor(ctr[:], src[:, t, csl], mb[:],
                                            op=OP.subtract)
                    nc.vector.tensor_tensor(sln[:, t, :], ctr[:], rb[:],
                                            op=OP.mult)
                    if use_g1:
                        nc.vector.tensor_scalar(sln[:, t, :], sln[:, t, :],
                                                g1_c[:, t, :], be1_c[:, t, :],
                                                op0=OP.mult, op1=OP.add)
                hid = wc.tile([128, 6, CH], BF16, tag="hid")
                for mt in range(6):
                    hp = cps.tile([128, CH], F32, tag="cp")
                    for t in range(3):
                        nc.tensor.matmul(hp[:], w3[:, t, 128 * mt:128 * (mt + 1)],
                                         sln[:, t, :],
                                         start=(t == 0), stop=(t == 2))
                    nc.scalar.activation(hid[:, mt, :], hp[:], AF.Gelu,
                                         bias=b3_c[:, mt, :], scale=1.0)
                return sln, hid

            def emit_c2a(ch, sln, hid):
                # W4 + residual + LN2 mean; squares go to gpsimd
                s2 = srp.tile([128, 3, CH], BF16, tag="s2", bufs=3)
                for t in range(3):
                    fp = cps.tile([128, CH], F32, tag="cp")
                    for kt in range(6):
                        nc.tensor.matmul(fp[:], w4[:, kt, 128 * t:128 * (t + 1)],
                                         hid[:, kt, :],
                                         start=(kt == 0), stop=(kt == 5))
                    nc.vector.scalar_tensor_tensor(
                        s2[:, t, :], fp[:], b4_c[:, t, :], sln[:, t, :],
                        op0=OP.add, op1=OP.add)
                stm2 = cps.tile([128, CH], F32, tag="cp")
                for t in range(3):
                    nc.tensor.matmul(stm2[0:1, :], onesdb[:], s2[:, t, :],
                                     start=(t == 0), stop=(t == 2))
                mrow = srp.tile([1, CH], F32, tag="mrow")
                nc.vector.tensor_copy(mrow[:], stm2[0:1, :])
                sq2l = []
                for t in range(3):
                    sq2 = wc.tile([128, CH], BF16, tag="sq2", bufs=6)
                    nc.vector.tensor_tensor(sq2[:], s2[:, t, :], s2[:, t, :],
                                            op=OP.mult)
                    sq2l.append(sq2)
                return s2, mrow, sq2l

            def emit_c2b(ch, s2, mrow, sq2l):
                sts2 = cps.tile([128, CH], F32, tag="cp")
                for t in range(3):
                    nc.tensor.matmul(sts2[0:1, :], onesdb[:], sq2l[t][:],
                                     start=(t == 0), stop=(t == 2))
                vrow = srp.tile([1, CH], F32, tag="vrow", bufs=2)
                nc.vector.tensor_tensor(vrow[:], mrow[:], mrow[:], op=OP.mult)
                nc.vector.tensor_tensor(vrow[:], sts2[0:1, :], vrow[:],
                                        op=OP.subtract)
                svr = srp.tile([1, CH], F32, tag="vrow", bufs=2)
                nc.scalar.activation(svr[:], vrow[:], AF.Sqrt,
                                     bias=eps_t[0:1, 0:1], scale=1.0)
                rrow = srp.tile([1, CH], F32, tag="rrow")
                nc.vector.reciprocal_approx_fast(rrow[:], svr[:])
                return s2, mrow, rrow

            ot_tiles = {}

            def emit_e(ch, s2, mrow, rrow):
                # LN2 apply via gpsimd broadcasts + store the chunk
                csl = slice(CH * ch, CH * (ch + 1))
                mb2 = srp.tile([128, CH], F32, tag="bc2")
                nc.gpsimd.partition_broadcast(mb2[:], mrow[:])
                rb2 = srp.tile([128, CH], F32, tag="bc2")
                nc.gpsimd.partition_broadcast(rb2[:], rrow[:])
                ot = srp.tile([128, 3, CH], F32, tag="ot")
                ot_tiles[ch] = ot
                for t in range(3):
                    ctr2 = wc.tile([128, CH], F32, tag="ctr2")
                    nc.vector.tensor_tensor(ctr2[:], s2[:, t, :], mb2[:],
                                            op=OP.subtract)
                    nc.vector.tensor_tensor(ot[:, t, :], ctr2[:], rb2[:],
                                            op=OP.mult)
                    if use_g2:
                        nc.vector.tensor_scalar(ot[:, t, :], ot[:, t, :],
                                                g2_c[:, t, :], be2_c[:, t, :],
                                                op0=OP.mult, op1=OP.add)
                nc.sync.dma_start(
                    OALL.rearrange("(t p) l -> p t l", p=128)[:, :, csl],
                    ot[:])
                # glb accumulation: even rows fully stored by now, in-place
                # add their even columns into gout (rows may straddle chunks)
                lim = CH * (ch + 1)
                r = emit_e.next_row
                while 96 * (r + 1) <= lim:
                    t0, t1 = 96 * r, 96 * (r + 1)
                    segs = []
                    cs = t0 // CH
                    if t1 - 1 >= CH * (cs + 1):
                        m = CH * (cs + 1)
                        segs = [(cs, t0, m), (cs + 1, m, t1)]
                    else:
                        segs = [(cs, t0, t1)]
                    for (c_, a, b) in segs:
                        k0, k1 = (a - t0) // 2, (b - t0) // 2
                        otc = ot_tiles[c_]
                        gsl = slice((r // 2) * 48 + k0, (r // 2) * 48 + k1)
                        for t in range(3):
                            ev = otc[:, t, a - CH * c_:b - CH * c_].rearrange(
                                "p (c two) -> p c two", two=2)[:, :, 0]
                            nc.vector.tensor_tensor(gout[:, t, gsl],
                                                    gout[:, t, gsl], ev,
                                                    op=OP.add)
                    r += 2
                emit_e.next_row = r
                if ch >= 2:
                    ot_tiles.pop(ch - 2, None)
            emit_e.next_row = 0

            stc, sta, ste = {}, {}, {}
            for ch in range(NCH):
                stc[ch] = emit_c1(ch)
                if ch >= 1:
                    sta[ch - 1] = emit_c2a(ch - 1, *stc.pop(ch - 1))
                if ch >= 2:
                    ste[ch - 2] = emit_c2b(ch - 2, *sta.pop(ch - 2))
                if ch >= 3:
                    emit_e(ch - 3, *ste.pop(ch - 3))
            sta[NCH - 1] = emit_c2a(NCH - 1, *stc.pop(NCH - 1))
            ste[NCH - 2] = emit_c2b(NCH - 2, *sta.pop(NCH - 2))
            emit_e(NCH - 3, *ste.pop(NCH - 3))
            ste[NCH - 1] = emit_c2b(NCH - 1, *sta.pop(NCH - 1))
            emit_e(NCH - 2, *ste.pop(NCH - 2))
            emit_e(NCH - 1, *ste.pop(NCH - 1))
            nc.sync.dma_start(
                OALL.rearrange("(t p) l -> p t l", p=128)[:, :, L:L + 1152],
                gout[:].bitcast(F32))

    nc.finalize()
    return nc


def _prep_core(x, sal_w, sal_b, attn_in_w, attn_in_b, attn_out_w, attn_out_b,
               w3, b3, w4, b4, core):
    n, hf = core // 2, core % 2
    hg, wg = n // 2, n % 2
    f = np.float32
    Xc = np.ascontiguousarray(
        x[n, :, 48 * hf:48 * hf + 48, :].reshape(D, L), dtype=f)
    PGc = np.ascontiguousarray(
        x[4, :, 48 * hg:48 * hg + 48, 48 * wg:48 * wg + 48].reshape(D, 2304)
    ).astype(mybir.dt.np(BF16))
    r0 = 48 * hg + 24 * hf
    GSc = np.ascontiguousarray(
        x[4, :, r0:r0 + 24, 48 * wg:48 * wg + 48].reshape(D, 1152), dtype=f)

    wq = attn_in_w[n, 0:D]
    wk = attn_in_w[n, D:2 * D]
    wv = attn_in_w[n, 2 * D:3 * D]
    bq = attn_in_b[n, 0:D]
    bk = attn_in_b[n, D:2 * D]
    bv = attn_in_b[n, 2 * D:3 * D]

    WQT = np.zeros((D, 512), f)
    WKT = np.zeros((D, 512), f)
    BQc = np.zeros((512, 1), f)
    BKc = np.zeros((512, 1), f)
    WVA = np.zeros((D + 1, 512), f)
    WOT = np.zeros((512, D), f)
    for h in range(NH):
        WQT[:, DP * h:DP * h + DH] = wq[DH * h:DH * (h + 1), :].T
        WKT[:, DP * h:DP * h + DH] = wk[DH * h:DH * (h + 1), :].T
        BQc[DP * h:DP * h + DH, 0] = bq[DH * h:DH * (h + 1)]
        BKc[DP * h:DP * h + DH, 0] = bk[DH * h:DH * (h + 1)]
        WVA[0:D, DP * h + 1:DP * h + 1 + DH] = wv[DH * h:DH * (h + 1), :].T
        WVA[D, DP * h + 1:DP * h + 1 + DH] = bv[DH * h:DH * (h + 1)]
        WVA[D, DP * h] = 1.0
        WOT[DP * h + 1:DP * h + 1 + DH, :] = attn_out_w[n, :, DH * h:DH * (h + 1)].T
    return {
        "X": Xc, "PG": PGc, "GS": GSc,
        "WQT": WQT, "WKT": WKT, "WVA": WVA,
        "WOT": np.ascontiguousarray(WOT).astype(mybir.dt.np(BF16)),
        "W3T": np.ascontiguousarray(w3.T, dtype=f),
        "W4T": np.ascontiguousarray(w4.T).astype(mybir.dt.np(BF16)),
        "SW": np.ascontiguousarray(sal_w.reshape(1, D).T, dtype=f),
        "ONESR": np.ones((1, 756), f),
        "ONESD": np.full((128, 1), 1.0 / D, f),
        "ONESDB": np.full((128, 1), 1.0 / D, mybir.dt.np(BF16)),
        "SEL9": _sel9(),
        "BQ": BQc, "BK": BKc,
        "BO": np.asarray(attn_out_b[n], f).reshape(D, 1),
        "B3": np.asarray(b3, f).reshape(768, 1),
        "B4": np.asarray(b4, f).reshape(D, 1),
        "G1": None, "BE1": None, "G2": None, "BE2": None,
        "SALB": np.asarray(sal_b, f).reshape(1, 1),
    }


def kernel(x, sal_w, sal_b, attn_in_w, attn_in_b, attn_out_w, attn_out_b,
           w3, b3, w4, b4, g1, be1, g2, be2):
    f = np.float32
    x = np.asarray(x, f)
    args = [np.asarray(a, f) for a in
            (sal_w, sal_b, attn_in_w, attn_in_b, attn_out_w, attn_out_b,
             w3, b3, w4, b4)]
    g1, be1, g2, be2 = (np.asarray(a, f) for a in (g1, be1, g2, be2))
    use_g1 = not (np.all(g1 == 1.0) and np.all(be1 == 0.0))
    use_g2 = not (np.all(g2 == 1.0) and np.all(be2 == 0.0))

    key = (use_g1, use_g2)
    if key not in _cache:
        _cache[key] = _build(use_g1, use_g2)
    nc = _cache[key]

    in_maps = []
    for core in range(8):
        m = _prep_core(x, *args, core)
        m["G1"] = g1.reshape(D, 1)
        m["BE1"] = be1.reshape(D, 1)
        m["G2"] = g2.reshape(D, 1)
        m["BE2"] = be2.reshape(D, 1)
        in_maps.append(m)

    res = run_bass_kernel_spmd(nc, in_maps, list(range(8)))

    out = np.empty((5, D, HW, HW), f)
    for core in range(8):
        n, hf = core // 2, core % 2
        hg, wg = n // 2, n % 2
        oall = res.results[core]["OALL"]
        osrc = oall[:, 0:L].reshape(D, 48, 96)
        out[n, :, 48 * hf:48 * hf + 48, :] = osrc
        oglb = oall[:, L:L + 1152].reshape(D, 24, 48)
        r0 = 48 * hg + 24 * hf
        out[4, :, r0:r0 + 24, 48 * wg:48 * wg + 48] = oglb
    return out



# revision 42
# speedup vs baseline: 145.4086x; 1.0116x over previous
"""Trainium2 Bass kernel for the MCRM block (4 local patches + global branch).

Sharding: 8 cores = 4 patches x 2 token-halves. Each core runs the full
attention+FFN pipeline for 4608 tokens of one patch in feature-major layout
(channels on partitions). The small global-branch work (saliency map, pooled
KV, glb output slice) is computed per-core on its slice.

Structure (per core, ~600 us HW time):
- setup: saliency map; 2x2 KV pooling with the row-pair stage folded into
  the PG load via an accumulating DMA; K/V projections. Chunk 0's X load,
  mask and Q projection are emitted early so the PE isn't idle behind the
  DVE pooling chain.
- phase A (attention), software-pipelined over 9 chunks of 512 tokens:
  per iteration the PE gets Q_ch | WO+stats(ch-2) | scores+AV_ch | sts(ch-2)
  so the softmax-normalize chain (reciprocal_approx_fast + gpsimd broadcast
  + multiply) of chunk ch resolves under the next chunk's matmuls and never
  stalls the PE. AV accumulates head PAIRS into one PSUM bank (tile_position
  column halves); the out-projection then contracts 128 rows per matmul.
  LN1 row stats stream to 16-partition tiles; one batched sqrt+recip at the
  end keeps the ACT Exp table resident all phase.
- phase C+E fused, software-pipelined: LN1 apply + W3+gelu (ch) | W4 +
  residual + LN2 stats (ch-1, per-chunk sqrt/fast-recip) | LN2 apply +
  per-chunk DRAM store (ch-2). The glb branch accumulates per-chunk into
  gout (straddling rows split into segments), so there is no full-L second
  buffer.

Each core emits ONE merged DRAM output OALL[D, L+1152] (src tokens in
columns [0:L], the glb slice in [L:L+1152]); extra output arrays cost a
full client round-trip each on the axon-tunneled execution path.
"""
import sys
sys.path.insert(0, '/opt/trn_rl_repo')
import numpy as np

import concourse.bacc as bacc
import concourse.mybir as mybir
import concourse.tile as tile
from concourse.bass_utils import run_bass_kernel_spmd

F32 = mybir.dt.float32
F32R = mybir.dt.float32r
BF16 = mybir.dt.bfloat16
AF = mybir.ActivationFunctionType
OP = mybir.AluOpType

D = 384          # d_model
NH = 8           # heads
DH = 48          # head dim
DP = 64          # padded head dim
HW = 96
L = 4608         # tokens per core (half patch)
CH = 512         # token chunk
NCH = L // CH    # 9
S = 756          # kv tokens (576+144+36)
NST = 6          # s-tiles of 126
ST = 126
SCALE = 1.0 / np.sqrt(48.0)
EPS = 1e-5

_cache = {}


def _sel9():
    s = np.zeros((16, NCH * 128), np.float32)
    for ch in range(NCH):
        s[ch, 128 * ch:128 * (ch + 1)] = 1.0
    return s


def _build(use_g1, use_g2):
    nc = bacc.Bacc(target_bir_lowering=False, debug=False)

    def dparam(name, shape, dt=F32R):
        return nc.declare_dram_parameter(name, list(shape), dt, isOutput=False)

    X = dparam("X", (D, L))
    PG = dparam("PG", (D, 2304), BF16)
    GS = dparam("GS", (D, 1152))
    WQT = dparam("WQT", (D, 512))
    WKT = dparam("WKT", (D, 512))
    WVA = dparam("WVA", (D + 1, 512))
    WOT = dparam("WOT", (512, D), BF16)
    W3T = dparam("W3T", (D, 768))
    W4T = dparam("W4T", (768, D), BF16)
    SW = dparam("SW", (D, 1))
    ONESR = dparam("ONESR", (1, 756))
    ONESD = dparam("ONESD", (128, 1))          # value 1/384
    ONESDB = dparam("ONESDB", (128, 1), BF16)  # value 1/384
    SEL9 = dparam("SEL9", (16, NCH * 128))     # one-hot row selectors
    BQ = dparam("BQ", (512, 1), F32)
    BK = dparam("BK", (512, 1), F32)
    BO = dparam("BO", (D, 1), F32)
    B3 = dparam("B3", (768, 1), F32)
    B4 = dparam("B4", (D, 1), F32)
    G1 = dparam("G1", (D, 1), F32)
    BE1 = dparam("BE1", (D, 1), F32)
    G2 = dparam("G2", (D, 1), F32)
    BE2 = dparam("BE2", (D, 1), F32)
    SALB = dparam("SALB", (1, 1), F32)
    # single merged output: columns [0:L] = src slice, [L:L+1152] = glb slice
    # (one output array per call: each extra output costs a full client
    # round-trip on the remote execution path)
    OALL = nc.declare_dram_parameter("OALL", [D, L + 1152], F32, isOutput=True)

    NP = 16  # partition count for stats tiles (>= NCH)

    with tile.TileContext(nc) as tc:
      with tc.tile_pool(name="const", bufs=1) as cp:
        onesd = cp.tile([128, 1], F32R)
        nc.sync.dma_start(onesd[:], ONESD[:])
        onesdb = cp.tile([128, 1], BF16)
        nc.sync.dma_start(onesdb[:], ONESDB[:])
        sel9 = cp.tile([NP, NCH * 128], F32R)
        nc.sync.dma_start(sel9[:], SEL9[:])
        salb = cp.tile([1, 1], F32)
        nc.sync.dma_start(salb[:], SALB[:])
        eps_t = cp.tile([NP, 1], F32)
        nc.vector.memset(eps_t[:], float(EPS))

        def col(par, n, tg):
            t = cp.tile([128, n // 128, 1], F32, tag=tg)
            nc.sync.dma_start(t[:], par.rearrange("(t p) o -> p t o", p=128))
            return t
        bq_c = col(BQ, 512, "bqc")
        bo_c = col(BO, D, "boc")
        b3_c = col(B3, 768, "b3c")
        b4_c = col(B4, D, "b4c")
        g1_c = col(G1, D, "g1c")
        be1_c = col(BE1, D, "be1c")
        g2_c = col(G2, D, "g2c")
        be2_c = col(BE2, D, "be2c")

        # per-chunk LN stats, one row per chunk (partition = chunk index)
        m19 = cp.tile([NP, CH], F32R)
        rs19 = cp.tile([NP, CH], F32R)
        rq19 = cp.tile([NP, CH], F32R)
        m29 = cp.tile([NP, CH], F32R)
        rs29 = cp.tile([NP, CH], F32R)
        tamup = cp.tile([1, L], F32)
        for _s in (m19, rs19, rq19, m29, rs29):
            nc.vector.memset(_s[:].bitcast(F32), 0.0)

        with tc.tile_pool(name="bigA", bufs=1) as bpa:
          src = bpa.tile([128, 3, L], F32R, tag="bigA")
          with tc.tile_pool(name="apool", bufs=1) as apl:
            wq = apl.tile([128, 3, 512], F32R)
            nc.sync.dma_start(wq[:], WQT.rearrange("(t p) m -> p t m", p=128))
            wo = apl.tile([128, 4, D], BF16)
            nc.sync.dma_start(wo[:], WOT.rearrange("(h p) m -> p h m", p=128))
            k_sb = apl.tile([128, 4, S], F32R)
            v_sb = apl.tile([126, 6, 512], BF16)

            # ------------- setup: tam, pooling, K/V proj -------------
            with tc.tile_pool(name="setup", bufs=1) as spl, \
                 tc.tile_pool(name="setps", bufs=2, space="PSUM") as sps:
                wk = spl.tile([128, 3, 512], F32R)
                nc.sync.dma_start(wk[:], WKT.rearrange("(t p) m -> p t m", p=128))
                wv = spl.tile([128, 3, 512], F32R)
                nc.sync.dma_start(wv[:], WVA[0:D, :].rearrange("(t p) m -> p t m", p=128))
                wv1 = spl.tile([1, 512], F32R)
                nc.sync.dma_start(wv1[:], WVA[D:D + 1, :])
                sw = spl.tile([128, 3, 1], F32R)
                nc.sync.dma_start(sw[:], SW.rearrange("(t p) o -> p t o", p=128))
                onesr = spl.tile([1, 756], F32R)
                nc.sync.dma_start(onesr[:], ONESR[:])
                bk_c = spl.tile([128, 4, 1], F32)
                nc.sync.dma_start(bk_c[:], BK.rearrange("(t p) o -> p t o", p=128))
                gs = spl.tile([128, 3, 1152], F32R)
                nc.sync.dma_start(gs[:], GS.rearrange("(t p) m -> p t m", p=128))
                pg = spl.tile([128, 3, 2304], BF16)
                nc.sync.dma_start(pg[:], PG.rearrange("(t p) m -> p t m", p=128))

                uview = tamup[:].rearrange("o (r a c b) -> o r a c b", r=24, a=2, b=2)
                for nchk in range(3):
                    tp = sps.tile([1, 384], F32, tag="tamps")
                    for t in range(3):
                        nc.tensor.matmul(tp[:], sw[:, t, :],
                                         gs[:, t, 384 * nchk:384 * (nchk + 1)],
                                         start=(t == 0), stop=(t == 2))
                    tamc = spl.tile([1, 384], F32, tag="tamc", bufs=2,
                                    name=f"tamc_{nchk}")
                    nc.scalar.activation(tamc[:], tp[:],
                                         AF.Sigmoid, bias=salb[:, 0:1], scale=1.0)
                    tcv = tamc[:].rearrange("o (r c) -> o r c", r=8)
                    for a in range(2):
                        for b in range(2):
                            nc.vector.tensor_copy(
                                uview[:, 8 * nchk:8 * (nchk + 1), a, :, b], tcv[:])

                kv = spl.tile([128, 3, S], F32R)
                for t in range(3):
                    pgv = pg[:, t, :].rearrange("p (r a c) -> p r a c", r=24, a=2)
                    h1 = spl.tile([128, 24, 48], F32, tag="poolh")
                    nc.vector.tensor_tensor(h1[:], pgv[:, :, 0, :], pgv[:, :, 1, :],
                                            op=OP.add)
                    h1v = h1[:].rearrange("p r (c d) -> p r c d", d=2)
                    p4 = spl.tile([128, 24, 24], F32, tag="poolp4")
                    nc.vector.tensor_tensor(p4[:], h1v[:, :, :, 0], h1v[:, :, :, 1],
                                            op=OP.add)
                    nc.vector.tensor_scalar(kv[:, t, 0:576].rearrange(
                        "p (r c) -> p r c", r=24), p4[:], 0.25, None, op0=OP.mult)
                    k4 = kv[:, t, 0:576].rearrange("p (r a c) -> p r a c", r=12, a=2)
                    h2 = spl.tile([128, 12, 24], F32, tag="poolh2")
                    nc.vector.tensor_tensor(h2[:], k4[:, :, 0, :], k4[:, :, 1, :],
                                            op=OP.add)
                    h2v = h2[:].rearrange("p r (c d) -> p r c d", d=2)
                    p8 = spl.tile([128, 12, 12], F32, tag="poolp8")
                    nc.vector.tensor_tensor(p8[:], h2v[:, :, :, 0], h2v[:, :, :, 1],
                                            op=OP.add)
                    nc.vector.tensor_scalar(kv[:, t, 576:720].rearrange(
                        "p (r c) -> p r c", r=12), p8[:], 0.25, None, op0=OP.mult)
                    k8 = kv[:, t, 576:720].rearrange("p (r a c) -> p r a c", r=6, a=2)
                    h3 = spl.tile([128, 6, 12], F32, tag="poolh3")
                    nc.vector.tensor_tensor(h3[:], k8[:, :, 0, :], k8[:, :, 1, :],
                                            op=OP.add)
                    h3v = h3[:].rearrange("p r (c d) -> p r c d", d=2)
                    p16 = spl.tile([128, 6, 6], F32, tag="poolp16")
                    nc.vector.tensor_tensor(p16[:], h3v[:, :, :, 0], h3v[:, :, :, 1],
                                            op=OP.add)
                    nc.vector.tensor_scalar(kv[:, t, 720:756].rearrange(
                        "p (r c) -> p r c", r=6), p16[:], 0.25, None, op0=OP.mult)

                for mt in range(4):
                    for nchk in range(2):
                        nsl = slice(378 * nchk, 378 * (nchk + 1))
                        kp = sps.tile([128, 378], F32, tag="kvps")
                        for t in range(3):
                            nc.tensor.matmul(kp[:], wk[:, t, 128 * mt:128 * (mt + 1)],
                                             kv[:, t, nsl],
                                             start=(t == 0), stop=(t == 2))
                        nc.vector.tensor_scalar(k_sb[:, mt, nsl], kp[:],
                                                bk_c[:, mt, :], None, op0=OP.add)
                for st in range(NST):
                    ssl = slice(ST * st, ST * (st + 1))
                    vp = sps.tile([126, 512], F32, tag="kvps")
                    for t in range(3):
                        nc.tensor.matmul(vp[:], kv[:, t, ssl], wv[:, t, :],
                                         start=(t == 0), stop=False)
                    nc.tensor.matmul(vp[:], onesr[:, ssl], wv1[:],
                                     start=False, stop=True)
                    nc.vector.tensor_copy(v_sb[:, st, :], vp[:])

            # ------------- phase A: attention, software-pipelined -------------
            # Per iteration ch the engine queues get:
            #   PE : Q_ch | WO+stats(ch-2) | scores+AV_ch | sts(ch-2)
            #   DVE: mask_{ch+1} qbias_ch residual/sq_{ch-2} norm_ch stats_{ch-2}
            # so the softmax-normalize chain (recip+bcast+mult) of chunk ch
            # resolves during the next chunk's scores/AV and never stalls PE.
            with tc.tile_pool(name="wka", bufs=4) as wa, \
                 tc.tile_pool(name="tbp", bufs=2) as tbp, \
                 tc.tile_pool(name="qpool", bufs=4) as qpl, \
                 tc.tile_pool(name="epool", bufs=6) as ep, \
                 tc.tile_pool(name="aow", bufs=2) as aw, \
                 tc.tile_pool(name="aonp", bufs=9) as awn, \
                 tc.tile_pool(name="sqp", bufs=2) as sqp, \
                 tc.tile_pool(name="bigps", bufs=2, space="PSUM") as bps, \
                 tc.tile_pool(name="avps", bufs=3, space="PSUM") as avp:

                def emit_xdma(ch):
                    csl = slice(CH * ch, CH * (ch + 1))
                    xm = wa.tile([128, 3, CH], F32R, tag="xc")
                    nc.sync.dma_start(
                        xm[:], X.rearrange("(t p) l -> p t l", p=128)[:, :, csl])
                    return xm

                def emit_mask(ch, xm):
                    csl = slice(CH * ch, CH * (ch + 1))
                    tb = tbp.tile([128, CH], F32, tag="tb")
                    nc.gpsimd.partition_broadcast(tb[:], tamup[:, csl])
                    for t in range(3):
                        nc.vector.tensor_tensor(xm[:, t, :], xm[:, t, :], tb[:],
                                                op=OP.mult)

                def emit_q(ch, xm):
                    q = []
                    for mt in range(4):
                        qp = bps.tile([128, 2 * CH], F32, tag="ps")
                        for t in range(3):
                            nc.tensor.matmul(qp[:, 0:CH],
                                             wq[:, t, 128 * mt:128 * (mt + 1)],
                                             xm[:, t, :],
                                             start=(t == 0), stop=(t == 2))
                        qt = qpl.tile([128, CH], F32R, tag="q")
                        nc.vector.tensor_scalar(qt[:], qp[:, 0:CH],
                                                bq_c[:, mt, :], None, op0=OP.add)
                        q.append(qt)
                    return q

                def emit_attn(ch, q):
                    aon = []
                    for mt in range(4):          # head pair (heads 2mt, 2mt+1)
                        e_t = []
                        for st in range(NST):
                            ssl = slice(ST * st, ST * (st + 1))
                            sp = bps.tile([128, 2 * CH], F32, tag="ps")
                            nc.tensor.matmul(
                                sp[0:126, 0:CH],
                                k_sb[0:64, mt, ssl], q[mt][0:64, :],
                                start=True, stop=True, tile_position=(0, 0))
                            nc.tensor.matmul(
                                sp[0:126, CH:2 * CH],
                                k_sb[64:128, mt, ssl], q[mt][64:128, :],
                                start=True, stop=True, tile_position=(64, 0))
                            et = ep.tile([126, 2 * CH], BF16, tag="e")
                            nc.scalar.activation(et[:], sp[0:126, :],
                                                 AF.Exp, scale=float(SCALE))
                            e_t.append(et)
                        # both heads of the pair accumulate into one PSUM bank:
                        # rows 0:64 head 2mt, rows 64:128 head 2mt+1
                        ap_ = avp.tile([128, CH], F32, tag="av")
                        for st in range(NST):
                            nc.tensor.matmul(
                                ap_[0:64, :],
                                v_sb[:, st, 128 * mt:128 * mt + 64],
                                e_t[st][:, 0:CH],
                                start=(st == 0), stop=(st == NST - 1),
                                tile_position=(0, 0))
                            nc.tensor.matmul(
                                ap_[64:128, :],
                                v_sb[:, st, 128 * mt + 64:128 * (mt + 1)],
                                e_t[st][:, CH:2 * CH],
                                start=(st == 0), stop=(st == NST - 1),
                                tile_position=(0, 64))
                        rc0 = aw.tile([1, CH], F32, tag="rowtmp")
                        nc.vector.reciprocal_approx_fast(rc0[:], ap_[0:1, :])
                        rc1 = aw.tile([1, CH], F32, tag="rowtmp")
                        nc.vector.reciprocal_approx_fast(rc1[:], ap_[64:65, :])
                        # partition_broadcast only lands correctly on ranges
                        # based at partition 0 — broadcast each denominator to
                        # a full tile and multiply the matching lane halves
                        bca = aw.tile([128, CH], F32, tag="bc")
                        nc.gpsimd.partition_broadcast(bca[:], rc0[:])
                        bcb = aw.tile([128, CH], F32, tag="bc")
                        nc.gpsimd.partition_broadcast(bcb[:], rc1[:])
                        an = awn.tile([128, CH], BF16, tag="aon")
                        nc.vector.tensor_tensor(an[0:64, :], ap_[0:64, :],
                                                bca[0:64, :], op=OP.mult)
                        nc.vector.tensor_tensor(an[64:128, :], ap_[64:128, :],
                                                bcb[64:128, :], op=OP.mult)
                        aon.append(an)
                    return aon

                def emit_b1(ch, xm, aon):
                    # out-projection (contraction 128 = head pair) + residual
                    csl = slice(CH * ch, CH * (ch + 1))
                    for t in range(3):
                        op_ = bps.tile([128, 2 * CH], F32, tag="ps")
                        for mt in range(4):
                            nc.tensor.matmul(op_[:, 0:CH],
                                             wo[:, mt, 128 * t:128 * (t + 1)],
                                             aon[mt][:],
                                             start=(mt == 0), stop=(mt == 3))
                        nc.vector.scalar_tensor_tensor(
                            src[:, t, csl], op_[:, 0:CH], bo_c[:, t, :],
                            xm[:, t, :], op0=OP.add, op1=OP.add)
                    stm = bps.tile([128, 2 * CH], F32, tag="ps")
                    for t in range(3):
                        nc.tensor.matmul(stm[0:1, 0:CH], onesd[:], src[:, t, csl],
                                         start=(t == 0), stop=(t == 2))
                    mrow = aw.tile([1, CH], F32R, tag="mrow", bufs=2)
                    nc.vector.tensor_copy(mrow[:], stm[0:1, 0:CH])
                    nc.sync.dma_start(m19[ch:ch + 1, :], mrow[:])
                    sq = []
                    for t in range(3):
                        sqt = sqp.tile([128, CH], F32R, tag="sq", bufs=4)
                        nc.vector.tensor_tensor(sqt[:], src[:, t, csl],
                                                src[:, t, csl], op=OP.mult)
                        sq.append(sqt)
                    return sq

                def emit_b2(ch, sq):
                    sts = bps.tile([128, 2 * CH], F32, tag="ps")
                    for t in range(3):
                        nc.tensor.matmul(sts[0:1, 0:CH], onesd[:], sq[t][:],
                                         start=(t == 0), stop=(t == 2))
                    vrow = aw.tile([1, CH], F32R, tag="mrow", bufs=2)
                    nc.vector.tensor_copy(vrow[:], sts[0:1, 0:CH])
                    nc.sync.dma_start(rs19[ch:ch + 1, :], vrow[:])

                st_xm = {0: emit_xdma(0), 1: emit_xdma(1)}
                emit_mask(0, st_xm[0])
                st_aon, st_b = {}, {}
                for ch in range(NCH):
                    if ch + 2 < NCH:
                        st_xm[ch + 2] = emit_xdma(ch + 2)
                    if ch + 1 < NCH:
                        emit_mask(ch + 1, st_xm[ch + 1])
                    q = emit_q(ch, st_xm[ch])
                    if ch >= 2:
                        st_b[ch - 2] = emit_b1(ch - 2, st_xm[ch - 2],
                                               st_aon.pop(ch - 2))
                        del st_xm[ch - 2]
                    st_aon[ch] = emit_attn(ch, q)
                    if ch >= 3:
                        emit_b2(ch - 3, st_b.pop(ch - 3))
                stats_batch(NCH - 3)
                st_b[NCH - 2] = emit_b1(NCH - 2, st_xm.pop(NCH - 2),
                                        st_aon.pop(NCH - 2))
                emit_b2(NCH - 3, st_b.pop(NCH - 3))
                st_b[NCH - 1] = emit_b1(NCH - 1, st_xm.pop(NCH - 1),
                                        st_aon.pop(NCH - 1))
                emit_b2(NCH - 2, st_b.pop(NCH - 2))
                emit_b2(NCH - 1, st_b.pop(NCH - 1))

                # var = E[x^2] - mean^2 then 1/sqrt(var+eps) -> rq19;
                # rows [0, r1). rs19 keeps raw E[x^2] so the full-width
                # second pass can recompute rows 0:6 harmlessly.
                def stats_batch(r1):
                    msq = aw.tile([NP, CH], F32, tag="svall")
                    nc.vector.tensor_tensor(msq[0:r1, :], m19[0:r1, :],
                                            m19[0:r1, :], op=OP.mult)
                    nc.vector.tensor_tensor(msq[0:r1, :],
                                            rs19[0:r1, :].bitcast(F32),
                                            msq[0:r1, :], op=OP.subtract)
                    sva = aw.tile([NP, CH], F32, tag="svall")
                    nc.scalar.activation(sva[0:r1, :], msq[0:r1, :], AF.Sqrt,
                                         bias=eps_t[0:r1, 0:1], scale=1.0)
                    svb = aw.tile([NP, CH], F32, tag="svall")
                    nc.vector.reciprocal_approx_fast(svb[0:r1, :],
                                                     sva[0:r1, :])
                    nc.vector.tensor_copy(rq19[0:r1, :], svb[0:r1, :])
                stats_batch(NP)

          # ------- phase C+E fused: LN1+FFN+LN2+store, software-pipelined -------
          # Per iteration: PE gets mb/rb+W3(ch) | W4+stats2(ch-1); the LN2 of
          # chunk ch-2 (per-chunk sqrt/recip + gpsimd broadcasts + DVE apply)
          # resolves under the next chunks' matmuls. Output slices stream to
          # DRAM per chunk; the glb branch accumulates per-chunk into gout.
          with tc.tile_pool(name="cpool", bufs=1) as cpl, \
               tc.tile_pool(name="wkc", bufs=2) as wc, \
               tc.tile_pool(name="srmurow", bufs=3) as srp, \
               tc.tile_pool(name="cps", bufs=6, space="PSUM") as cps, \
               tc.tile_pool(name="cbc", bufs=2, space="PSUM") as cbc:
            w3 = cpl.tile([128, 3, 768], F32R)
            nc.sync.dma_start(w3[:], W3T.rearrange("(t p) m -> p t m", p=128))
            w4 = cpl.tile([128, 6, D], BF16)
            nc.sync.dma_start(w4[:], W4T.rearrange("(t p) m -> p t m", p=128))
            gout = cpl.tile([128, 3, 1152], F32R)
            nc.sync.dma_start(gout[:], GS.rearrange("(t p) m -> p t m", p=128))

            def emit_c1(ch):
                # LN1 apply + W3 + gelu
                csl = slice(CH * ch, CH * (ch + 1))
                nr = NCH - 3 if ch < NCH - 3 else NP
                ssel = sel9[0:nr, 128 * ch:128 * (ch + 1)]
                mb = cbc.tile([128, CH], F32, tag="cbc")
                nc.tensor.matmul(mb[:], ssel, m19[0:nr, :],
                                 start=True, stop=True)
                rb = cbc.tile([128, CH], F32, tag="cbc")
                nc.tensor.matmul(rb[:], ssel, rq19[0:nr, :],
                                 start=True, stop=True)
                sln = wc.tile([128, 3, CH], F32R, tag="sln")
                for t in range(3):
                    ctr = wc.tile([128, CH], F32, tag="ctr")
                    nc.vector.tensor_tensor(ctr[:], src[:, t, csl], mb[:],
                                            op=OP.subtract)
                    nc.vector.tensor_tensor(sln[:, t, :], ctr[:], rb[:],
                                            op=OP.mult)
                    if use_g1:
                        nc.vector.tensor_scalar(sln[:, t, :], sln[:, t, :],
                                                g1_c[:, t, :], be1_c[:, t, :],
                                                op0=OP.mult, op1=OP.add)
                hid = wc.tile([128, 6, CH], BF16, tag="hid")
                for mt in range(6):
                    hp = cps.tile([128, CH], F32, tag="cp")
                    for t in range(3):
                        nc.tensor.matmul(hp[:], w3[:, t, 128 * mt:128 * (mt + 1)],
                                         sln[:, t, :],
                                         start=(t == 0), stop=(t == 2))
                    nc.scalar.activation(hid[:, mt, :], hp[:], AF.Gelu,
                                         bias=b3_c[:, mt, :], scale=1.0)
                return sln, hid

            def emit_c2a(ch, sln, hid):
                # W4 + residual + LN2 mean; squares go to gpsimd
                s2 = srp.tile([128, 3, CH], BF16, tag="s2", bufs=3)
                for t in range(3):
                    fp = cps.tile([128, CH], F32, tag="cp")
                    for kt in range(6):
                        nc.tensor.matmul(fp[:], w4[:, kt, 128 * t:128 * (t + 1)],
                                         hid[:, kt, :],
                                         start=(kt == 0), stop=(kt == 5))
                    nc.vector.scalar_tensor_tensor(
                        s2[:, t, :], fp[:], b4_c[:, t, :], sln[:, t, :],
                        op0=OP.add, op1=OP.add)
                stm2 = cps.tile([128, CH], F32, tag="cp")
                for t in range(3):
                    nc.tensor.matmul(stm2[0:1, :], onesdb[:], s2[:, t, :],
                                     start=(t == 0), stop=(t == 2))
                mrow = srp.tile([1, CH], F32, tag="mrow")
                nc.vector.tensor_copy(mrow[:], stm2[0:1, :])
                sq2l = []
                for t in range(3):
                    sq2 = wc.tile([128, CH], BF16, tag="sq2", bufs=6)
                    nc.vector.tensor_tensor(sq2[:], s2[:, t, :], s2[:, t, :],
                                            op=OP.mult)
                    sq2l.append(sq2)
                return s2, mrow, sq2l

            def emit_c2b(ch, s2, mrow, sq2l):
                sts2 = cps.tile([128, CH], F32, tag="cp")
                for t in range(3):
                    nc.tensor.matmul(sts2[0:1, :], onesdb[:], sq2l[t][:],
                                     start=(t == 0), stop=(t == 2))
                vrow = srp.tile([1, CH], F32, tag="vrow", bufs=2)
                nc.vector.tensor_tensor(vrow[:], mrow[:], mrow[:], op=OP.mult)
                nc.vector.tensor_tensor(vrow[:], sts2[0:1, :], vrow[:],
                                        op=OP.subtract)
                svr = srp.tile([1, CH], F32, tag="vrow", bufs=2)
                nc.scalar.activation(svr[:], vrow[:], AF.Sqrt,
                                     bias=eps_t[0:1, 0:1], scale=1.0)
                rrow = srp.tile([1, CH], F32, tag="rrow")
                nc.vector.reciprocal_approx_fast(rrow[:], svr[:])
                return s2, mrow, rrow

            ot_tiles = {}

            def emit_e(ch, s2, mrow, rrow):
                # LN2 apply via gpsimd broadcasts + store the chunk
                csl = slice(CH * ch, CH * (ch + 1))
                mb2 = srp.tile([128, CH], F32, tag="bc2")
                nc.gpsimd.partition_broadcast(mb2[:], mrow[:])
                rb2 = srp.tile([128, CH], F32, tag="bc2")
                nc.gpsimd.partition_broadcast(rb2[:], rrow[:])
                ot = srp.tile([128, 3, CH], F32, tag="ot")
                ot_tiles[ch] = ot
                for t in range(3):
                    ctr2 = wc.tile([128, CH], F32, tag="ctr2")
                    nc.vector.tensor_tensor(ctr2[:], s2[:, t, :], mb2[:],
                                            op=OP.subtract)
                    nc.vector.tensor_tensor(ot[:, t, :], ctr2[:], rb2[:],
                                            op=OP.mult)
                    if use_g2:
                        nc.vector.tensor_scalar(ot[:, t, :], ot[:, t, :],
                                                g2_c[:, t, :], be2_c[:, t, :],
                                                op0=OP.mult, op1=OP.add)
                nc.sync.dma_start(
                    OALL.rearrange("(t p) l -> p t l", p=128)[:, :, csl],
                    ot[:])
                # glb accumulation: even rows fully stored by now, in-place
                # add their even columns into gout (rows may straddle chunks)
                lim = CH * (ch + 1)
                r = emit_e.next_row
                while 96 * (r + 1) <= lim:
                    t0, t1 = 96 * r, 96 * (r + 1)
                    segs = []
                    cs = t0 // CH
                    if t1 - 1 >= CH * (cs + 1):
                        m = CH * (cs + 1)
                        segs = [(cs, t0, m), (cs + 1, m, t1)]
                    else:
                        segs = [(cs, t0, t1)]
                    for (c_, a, b) in segs:
                        k0, k1 = (a - t0) // 2, (b - t0) // 2
                        otc = ot_tiles[c_]
                        gsl = slice((r // 2) * 48 + k0, (r // 2) * 48 + k1)
                        for t in range(3):
                            ev = otc[:, t, a - CH * c_:b - CH * c_].rearrange(
                                "p (c two) -> p c two", two=2)[:, :, 0]
                            nc.vector.tensor_tensor(gout[:, t, gsl],
                                                    gout[:, t, gsl], ev,
                                                    op=OP.add)
                    r += 2
                emit_e.next_row = r
                if ch >= 2:
                    ot_tiles.pop(ch - 2, None)
            emit_e.next_row = 0

            stc, sta, ste = {}, {}, {}
            for ch in range(NCH):
                stc[ch] = emit_c1(ch)
                if ch >= 1:
                    sta[ch - 1] = emit_c2a(ch - 1, *stc.pop(ch - 1))
                if ch >= 2:
                    ste[ch - 2] = emit_c2b(ch - 2, *sta.pop(ch - 2))
                if ch >= 3:
                    emit_e(ch - 3, *ste.pop(ch - 3))
            sta[NCH - 1] = emit_c2a(NCH - 1, *stc.pop(NCH - 1))
            ste[NCH - 2] = emit_c2b(NCH - 2, *sta.pop(NCH - 2))
            emit_e(NCH - 3, *ste.pop(NCH - 3))
            ste[NCH - 1] = emit_c2b(NCH - 1, *sta.pop(NCH - 1))
            emit_e(NCH - 2, *ste.pop(NCH - 2))
            emit_e(NCH - 1, *ste.pop(NCH - 1))
            nc.sync.dma_start(
                OALL.rearrange("(t p) l -> p t l", p=128)[:, :, L:L + 1152],
                gout[:].bitcast(F32))

    nc.finalize()
    return nc


def _prep_core(x, sal_w, sal_b, attn_in_w, attn_in_b, attn_out_w, attn_out_b,
               w3, b3, w4, b4, core):
    n, hf = core // 2, core % 2
    hg, wg = n // 2, n % 2
    f = np.float32
    Xc = np.ascontiguousarray(
        x[n, :, 48 * hf:48 * hf + 48, :].reshape(D, L), dtype=f)
    PGc = np.ascontiguousarray(
        x[4, :, 48 * hg:48 * hg + 48, 48 * wg:48 * wg + 48].reshape(D, 2304)
    ).astype(mybir.dt.np(BF16))
    r0 = 48 * hg + 24 * hf
    GSc = np.ascontiguousarray(
        x[4, :, r0:r0 + 24, 48 * wg:48 * wg + 48].reshape(D, 1152), dtype=f)

    wq = attn_in_w[n, 0:D]
    wk = attn_in_w[n, D:2 * D]
    wv = attn_in_w[n, 2 * D:3 * D]
    bq = attn_in_b[n, 0:D]
    bk = attn_in_b[n, D:2 * D]
    bv = attn_in_b[n, 2 * D:3 * D]

    WQT = np.zeros((D, 512), f)
    WKT = np.zeros((D, 512), f)
    BQc = np.zeros((512, 1), f)
    BKc = np.zeros((512, 1), f)
    WVA = np.zeros((D + 1, 512), f)
    WOT = np.zeros((512, D), f)
    for h in range(NH):
        WQT[:, DP * h:DP * h + DH] = wq[DH * h:DH * (h + 1), :].T
        WKT[:, DP * h:DP * h + DH] = wk[DH * h:DH * (h + 1), :].T
        BQc[DP * h:DP * h + DH, 0] = bq[DH * h:DH * (h + 1)]
        BKc[DP * h:DP * h + DH, 0] = bk[DH * h:DH * (h + 1)]
        WVA[0:D, DP * h + 1:DP * h + 1 + DH] = wv[DH * h:DH * (h + 1), :].T
        WVA[D, DP * h + 1:DP * h + 1 + DH] = bv[DH * h:DH * (h + 1)]
        WVA[D, DP * h] = 1.0
        WOT[DP * h + 1:DP * h + 1 + DH, :] = attn_out_w[n, :, DH * h:DH * (h + 1)].T
    return {
        "X": Xc, "PG": PGc, "GS": GSc,
        "WQT": WQT, "WKT": WKT, "WVA": WVA,
        "WOT": np.ascontiguousarray(WOT).astype(mybir.dt.np(BF16)),
        "W3T": np.ascontiguousarray(w3.T, dtype=f),
        "W4T": np.ascontiguousarray(w4.T).astype(mybir.dt.np(BF16)),
        "SW": np.ascontiguousarray(sal_w.reshape(1, D).T, dtype=f),
        "ONESR": np.ones((1, 756), f),
        "ONESD": np.full((128, 1), 1.0 / D, f),
        "ONESDB": np.full((128, 1), 1.0 / D, mybir.dt.np(BF16)),
        "SEL9": _sel9(),
        "BQ": BQc, "BK": BKc,
        "BO": np.asarray(attn_out_b[n], f).reshape(D, 1),
        "B3": np.asarray(b3, f).reshape(768, 1),
        "B4": np.asarray(b4, f).reshape(D, 1),
        "G1": None, "BE1": None, "G2": None, "BE2": None,
        "SALB": np.asarray(sal_b, f).reshape(1, 1),
    }


def kernel(x, sal_w, sal_b, attn_in_w, attn_in_b, attn_out_w, attn_out_b,
           w3, b3, w4, b4, g1, be1, g2, be2):
    f = np.float32
    x = np.asarray(x, f)
    args = [np.asarray(a, f) for a in
            (sal_w, sal_b, attn_in_w, attn_in_b, attn_out_w, attn_out_b,
             w3, b3, w4, b4)]
    g1, be1, g2, be2 = (np.asarray(a, f) for a in (g1, be1, g2, be2))
    use_g1 = not (np.all(g1 == 1.0) and np.all(be1 == 0.0))
    use_g2 = not (np.all(g2 == 1.0) and np.all(be2 == 0.0))

    key = (use_g1, use_g2)
    if key not in _cache:
        _cache[key] = _build(use_g1, use_g2)
    nc = _cache[key]

    in_maps = []
    for core in range(8):
        m = _prep_core(x, *args, core)
        m["G1"] = g1.reshape(D, 1)
        m["BE1"] = be1.reshape(D, 1)
        m["G2"] = g2.reshape(D, 1)
        m["BE2"] = be2.reshape(D, 1)
        in_maps.append(m)

    res = run_bass_kernel_spmd(nc, in_maps, list(range(8)))

    out = np.empty((5, D, HW, HW), f)
    for core in range(8):
        n, hf = core // 2, core % 2
        hg, wg = n // 2, n % 2
        oall = res.results[core]["OALL"]
        osrc = oall[:, 0:L].reshape(D, 48, 96)
        out[n, :, 48 * hf:48 * hf + 48, :] = osrc
        oglb = oall[:, L:L + 1152].reshape(D, 24, 48)
        r0 = 48 * hg + 24 * hf
        out[4, :, r0:r0 + 24, 48 * wg:48 * wg + 48] = oglb
    return out

